# revision 1
# baseline (speedup 1.0000x reference)
# MoE (top-2 of 8 experts) kernel for 8 Trainium2 NeuronCores.
#
# Strategy: expert-parallel sparse routing with fp8 DoubleRow matmuls.
# Host computes the gating network and per-expert token lists; core e runs
# expert e's FFN (x@W1+b1 -> LayerNorm -> erf-GELU -> @W2+b2) on its routed
# tokens. Both matmuls run as fp8(e4m3) DoubleRow (2 k-planes per
# instruction, 0.5 cyc/row = 4x the f32r MAC rate) with hi/lo error
# compensation: A@B ~= Ah@Bh + Al@Bh + Ah@Bl where Ah=fp8(A), Al=fp8(A-Ah).
# All weights live in SBUF (12MB fp8), loaded once in contiguous per-chunk
# DMAs. LayerNorm S-sums are folded into mm1 via a W1-column-sum lhsT row;
# Q-sums use a paired-fp8 DoubleRow ones-matmul on h^2; per-token stats
# broadcast across partitions with K=1 f32r matmuls, emitted mid-mm2 so the
# PE never waits on the stats chain. b2 is folded into the mm2 PSUM chain
# via a constant fp8 matmul so the mm2 evict is a single DVE
# tensor_scalar_mul (keeps ACT free for the GELUs). The first tile's
# normalize/GELU backlog is drip-fed through the second tile's mm1 loop.

import tempfile

import ml_dtypes
import numpy as np

import concourse.bacc as bacc
import concourse.mybir as mybir
import concourse.tile as tile
from concourse._compat import axon_active
from concourse.bass_utils import run_bass_kernel_spmd

P = 128
D, H, E, TOPK = 1024, 2048, 8, 2
DP, KP, NJ, NK = D // 256, H // 256, H // P, H // P  # 4, 8, 16, 16
LN_EPS = 1e-5
TT = 512           # main token tile
GRAN = 16          # capacity granularity
SX, SW1, SW2 = 16.0, 256.0, 256.0   # fp8 pre-quantization scales
SW1S = 32.0        # scale for the W1 column-sum row (S-fold)
F8 = ml_dtypes.float8_e4m3
BF = ml_dtypes.bfloat16

_kernel_cache: dict[int, object] = {}


def _t_tiles(C):
    tiles, t0 = [], 0
    while t0 < C:
        tt = TT if C - t0 >= TT else C - t0
        tiles.append((t0, tt))
        t0 += tt
    # Tail tile last: its cheap mm2 is the only un-overlapped one, and
    # full-size norm/GELU phases pair with full-size mm2 phases.
    return tiles


def _build(C: int):
    f32, f32r, bf16, f8 = (
        mybir.dt.float32, mybir.dt.float32r, mybir.dt.bfloat16, mybir.dt.float8e4
    )
    DR = mybir.MatmulPerfMode.DoubleRow
    Mul, Add = mybir.AluOpType.mult, mybir.AluOpType.add
    nc = bacc.Bacc("TRN2", target_bir_lowering=False, debug=False, num_devices=8)
    XH = nc.dram_tensor("XH", [P, DP, 2, C], f8, kind="ExternalInput").ap()
    XL = nc.dram_tensor("XL", [P, DP, 2, C], f8, kind="ExternalInput").ap()
    W1H = nc.dram_tensor("W1H", [P, NJ, DP, 2, P], f8, kind="ExternalInput").ap()
    W1L = nc.dram_tensor("W1L", [P, NJ, DP, 2, P], f8, kind="ExternalInput").ap()
    W1SH = nc.dram_tensor("W1SH", [P, DP, 2, 32], f8, kind="ExternalInput").ap()
    W1SL = nc.dram_tensor("W1SL", [P, DP, 2, 32], f8, kind="ExternalInput").ap()
    SB1H = nc.dram_tensor("SB1H", [1, 1], f32, kind="ExternalInput").ap()
    W2H = nc.dram_tensor("W2H", [P, NK, KP, 2, P], f8, kind="ExternalInput").ap()
    W2L = nc.dram_tensor("W2L", [P, NK, KP, 2, P], f8, kind="ExternalInput").ap()
    b1 = nc.dram_tensor("b1", [P, NJ], f32, kind="ExternalInput").ap()
    b2 = nc.dram_tensor("b2", [P, NK], f32, kind="ExternalInput").ap()
    lg = nc.dram_tensor("lg", [P, NJ], f32, kind="ExternalInput").ap()
    lb = nc.dram_tensor("lb", [P, NJ], f32, kind="ExternalInput").ap()
    outT = nc.dram_tensor("outT", [NK, P, C], bf16, kind="ExternalOutput").ap()

    Gelu = mybir.ActivationFunctionType.Gelu
    Sqrt = mybir.ActivationFunctionType.Sqrt
    Ident = mybir.ActivationFunctionType.Identity

    with tile.TileContext(nc) as tc:
        with (
            tc.tile_pool(name="const", bufs=1) as constp,
            tc.tile_pool(name="wp", bufs=1) as wp,
            tc.tile_pool(name="xp", bufs=1) as xp,
            tc.tile_pool(name="hp", bufs=2) as hp,
            tc.tile_pool(name="hxp", bufs=2) as hxp,
            tc.tile_pool(name="sqp", bufs=2) as sqp,
            tc.tile_pool(name="op", bufs=2) as op,
            tc.tile_pool(name="statp", bufs=1) as statp,
            tc.tile_pool(name="ps_mm", bufs=4, space="PSUM") as ps_mm,
            tc.tile_pool(name="ps_acc", bufs=1, space="PSUM") as ps_acc,
            tc.tile_pool(name="ps_bc", bufs=1, space="PSUM") as ps_bc,
        ):
            b1s = constp.tile([P, NJ], f32)
            b2s = constp.tile([P, NK], f32)
            lgs = constp.tile([P, NJ], f32)
            lbs = constp.tile([P, NJ], f32)
            w1sh = constp.tile([P, DP, 2, 32], f8)
            w1sl = constp.tile([P, DP, 2, 32], f8)
            sb1h = constp.tile([1, 1], f32)

            def emit_const_dmas():
                nc.sync.dma_start(b1s[:], b1[:])
                nc.sync.dma_start(lgs[:], lg[:])
                nc.sync.dma_start(lbs[:], lb[:])
                nc.sync.dma_start(w1sh[:], W1SH[:])
                nc.sync.dma_start(w1sl[:], W1SL[:])
                nc.sync.dma_start(sb1h[:], SB1H[:])

            ones_q = constp.tile([P, 2, 32], f8)   # lhsT for Q paired DR sums
            nc.any.memset(ones_q[:], 1.0)
            oner_f = constp.tile([1, P], f32)
            nc.any.memset(oner_f[:], 1.0)
            oner_c = constp.tile([1, P], f32r)     # lhsT for partition-broadcasts
            nc.vector.tensor_copy(oner_c[:], oner_f[:])
            eps_t = constp.tile([1, 1], f32)
            nc.any.memset(eps_t[:], LN_EPS)

            # fp8 weights, SBUF-resident for the whole kernel, streamed in
            # contiguous per-chunk DMAs staged around the first two tiles.
            w1h = wp.tile([P, NJ, DP, 2, P], f8)
            w1l = wp.tile([P, NJ, DP, 2, P], f8)
            w2h = wp.tile([P, NK, KP, 2, P], f8)
            w2l = wp.tile([P, NK, KP, 2, P], f8)

            def emit_w1(a, b):
                nc.sync.dma_start(w1h[:, a:b], W1H[:, a:b])
                nc.sync.dma_start(w1l[:, a:b], W1L[:, a:b])

            def emit_w2(a, b):
                nc.sync.dma_start(w2h[:, a:b], W2H[:, a:b])
                nc.sync.dma_start(w2l[:, a:b], W2L[:, a:b])

            def emit_mm2(h_hi, h_lo, t0, tt, mid=None, post=None):
                # 3-pass compensated fp8 mm2 + b2-fold, evict on DVE.
                # mid() runs after chain 5 (the next tile's stats-broadcast
                # matmuls); post(j) runs twice per chain from chain 6 (the
                # next tile's normalize/GELU work).
                step = 0
                for k in range(NK):
                    pm = ps_mm.tile([P, TT], f32, tag="mm", name="mm2")[:, :tt]
                    for pi, (wt, ht) in enumerate(
                        ((w2h, h_hi), (w2h, h_lo), (w2l, h_hi))
                    ):
                        for kp in range(KP):
                            nc.tensor.matmul(
                                pm[:],
                                wt[:, k, kp, :, :],
                                ht[:, kp, :, :tt],
                                start=(pi == 0 and kp == 0),
                                stop=(pi == 2 and kp == KP - 1),
                                perf_mode=DR,
                            )
                    ot = op.tile([P, tt], bf16, tag=f"out{tt}", name="out", bufs=4)
                    nc.vector.tensor_scalar(
                        ot[:], pm[:], 1.0 / SW2, b2s[:, k : k + 1], Mul, Add
                    )
                    nc.sync.dma_start(outT[k, :, t0 : t0 + tt], ot[:])
                    if k == 5 and mid is not None:
                        mid()
                    if k >= 6 and post is not None:
                        for _ in range(2):
                            if step < NJ:
                                post(step)
                                step += 1
                while post is not None and step < NJ:
                    post(step)
                    step += 1

            prev = None
            backlog = []
            tiles = _t_tiles(C)
            for tile_i, (t0, tt) in enumerate(tiles):
                if tile_i == 0:
                    emit_w1(0, 1)  # W1 j=0 ahead of x so the first chain starts fast
                xh = xp.tile([P, DP, 2, TT], f8, tag="xh", name="xh")
                xl = xp.tile([P, DP, 2, TT], f8, tag="xl", name="xl")
                nc.sync.dma_start(xh[:, :, :, :tt], XH[:, :, :, t0 : t0 + tt])
                nc.sync.dma_start(xl[:, :, :, :tt], XL[:, :, :, t0 : t0 + tt])
                if tile_i == 0:
                    emit_w1(1, 3)
                    emit_const_dmas()
                h = hp.tile(
                    [P, NJ, tt], bf16, tag=f"h{tt}", name="h",
                    bufs=(2 if tt == TT else 1),
                )
                h_hi = hxp.tile(
                    [P, KP, 2, tt], f8, tag=f"hh{tt}", name="h_hi",
                    bufs=(2 if tt == TT else 1),
                )
                h_lo = hxp.tile(
                    [P, KP, 2, tt], f8, tag=f"hl{tt}", name="h_lo",
                    bufs=(2 if tt == TT else 1),
                )
                s_ps = ps_acc.tile([32, TT], f32, tag="sacc", name="sacc")[:, :tt]
                q_ps = ps_acc.tile([32, TT], f32, tag="qacc", name="qacc")[:, :tt]

                # ---- mm1 (3-pass fp8 DR); Q ones-matmuls deferred one pair
                # so the PE never waits on the ACT evict / DVE square chain;
                # tile0's norm/GELU backlog drip-fed through tile1's loop ----
                pend_q = None
                sq = None
                for j in range(NJ):
                    if tile_i == 0:
                        if j == 0:
                            emit_w1(3, 8)
                        elif j == 4:
                            emit_w1(8, NJ)
                        elif j == 8:
                            emit_w2(0, 4)
                        elif j == 12:
                            emit_w2(4, 8)
                    elif tile_i == 1:
                        if j == 0:
                            emit_w2(8, 12)
                        elif j == 4:
                            emit_w2(12, NK)
                        elif j == 8:
                            nc.sync.dma_start(b2s[:], b2[:])
                    pm = ps_mm.tile([P, TT], f32, tag="mm", name="mm1")[:, :tt]
                    for pi, (wt, xt) in enumerate(((w1h, xh), (w1h, xl), (w1l, xh))):
                        for dp in range(DP):
                            nc.tensor.matmul(
                                pm[:],
                                wt[:, j, dp, :, :],
                                xt[:, dp, :, :tt],
                                start=(pi == 0 and dp == 0),
                                stop=(pi == 2 and dp == DP - 1),
                                perf_mode=DR,
                            )
                    nc.scalar.activation(
                        h[:, j, :], pm[:], Ident,
                        bias=b1s[:, j : j + 1], scale=1.0 / (SX * SW1),
                    )
                    if j % 2 == 0:
                        sq = sqp.tile([P, 2, TT], f8, tag="sq", name="sq")
                    nc.vector.tensor_mul(sq[:, j % 2, :tt], h[:, j, :], h[:, j, :])
                    if j % 2 == 1:
                        if pend_q is not None:
                            jp, sqt = pend_q
                            nc.tensor.matmul(
                                q_ps[:], ones_q[:], sqt[:, :, :tt],
                                start=(jp == 0), stop=(jp == NJ // 2 - 1),
                                perf_mode=DR,
                            )
                        pend_q = (j // 2, sq)
                    if backlog:
                        backlog.pop(0)()
                while backlog:  # finish tile0's backlog before mm2(0) reads h
                    backlog.pop(0)()
                # S-fold: the W1-column-sum row, 3-pass DR into s_ps
                for pi, (wt, xt) in enumerate(((w1sh, xh), (w1sh, xl), (w1sl, xh))):
                    for dp in range(DP):
                        nc.tensor.matmul(
                            s_ps[:],
                            wt[:, dp, :, :],
                            xt[:, dp, :, :tt],
                            start=(pi == 0 and dp == 0),
                            stop=(pi == 2 and dp == DP - 1),
                            perf_mode=DR,
                        )
                jp, sqt = pend_q
                nc.tensor.matmul(
                    q_ps[:], ones_q[:], sqt[:, :, :tt],
                    start=(jp == 0), stop=(jp == NJ // 2 - 1),
                    perf_mode=DR,
                )

                # ---- LN stats (DVE/ACT only; broadcasts happen mid-mm2) ----
                mu = statp.tile([1, TT], f32, tag="mu", name="mu")[:, :tt]
                nc.vector.tensor_scalar(
                    mu[:], s_ps[0:1, :], 1.0 / (SX * SW1S * H), sb1h[:], Mul, Add
                )
                tmp = statp.tile([1, TT], f32, tag="tmp", name="tmp")[:, :tt]
                nc.vector.tensor_scalar_mul(tmp[:], q_ps[0:1, :], 1.0 / H)
                tmp2 = statp.tile([1, TT], f32, tag="tmp2", name="tmp2")[:, :tt]
                nc.vector.tensor_mul(tmp2[:], mu[:], mu[:])
                nc.vector.tensor_sub(tmp[:], tmp[:], tmp2[:])          # var
                nc.scalar.activation(tmp2[:], tmp[:], Sqrt, bias=eps_t[:])  # std
                nc.vector.reciprocal(tmp[:], tmp2[:])                  # rstd
                a_row = statp.tile([1, TT], f32r, tag="a_row", name="a_row")[:, :tt]
                nc.vector.tensor_copy(a_row[:], tmp[:])
                b_row = statp.tile([1, TT], f32r, tag="b_row", name="b_row")[:, :tt]
                nc.vector.tensor_mul(b_row[:], mu[:], tmp[:])

                a_sb = statp.tile([P, TT], bf16, tag="a_sb", name="a_sb", bufs=2)
                b_sb = statp.tile([P, TT], bf16, tag="b_sb", name="b_sb", bufs=2)

                def emit_bc(a_row=a_row, b_row=b_row, a_sb=a_sb, b_sb=b_sb, tt=tt):
                    a_bc = ps_bc.tile([P, TT], f32, tag="a_bc", name="a_bc")[:, :tt]
                    nc.tensor.matmul(
                        a_bc[:], oner_c[:], a_row[:], start=True, stop=True
                    )
                    b_bc = ps_bc.tile([P, TT], f32, tag="b_bc", name="b_bc")[:, :tt]
                    nc.tensor.matmul(
                        b_bc[:], oner_c[:], b_row[:], start=True, stop=True
                    )
                    nc.vector.tensor_copy(a_sb[:, :tt], a_bc[:])
                    nc.vector.tensor_copy(b_sb[:, :tt], b_bc[:])

                def emit_norm_gelu(
                    j, h=h, h_hi=h_hi, h_lo=h_lo, a_sb=a_sb, b_sb=b_sb, tt=tt,
                    spread=False,
                ):
                    # normalize (DVE bf16) + GELU twice (ACT: fp8 h_hi, bf16
                    # in place) + h_lo residual (DVE). In spread mode (tile0
                    # backlog, no mm2 window to hide in) the second GELU
                    # becomes a Pool copy and h_lo subs alternate Pool/DVE so
                    # the work balances across all three engines.
                    jp, pl = j // 2, j % 2
                    hj = h[:, j, :tt]
                    nc.vector.tensor_mul(hj, hj, a_sb[:, :tt])
                    nc.vector.tensor_sub(hj, hj, b_sb[:, :tt])
                    if spread:
                        nc.scalar.activation(
                            hj, hj, Gelu,
                            bias=lbs[:, j : j + 1], scale=lgs[:, j : j + 1],
                        )
                        nc.gpsimd.tensor_copy(h_hi[:, jp, pl, :tt], hj)
                        eng = nc.gpsimd if j % 2 else nc.vector
                        eng.tensor_sub(
                            h_lo[:, jp, pl, :tt], hj, h_hi[:, jp, pl, :tt]
                        )
                        return
                    nc.scalar.activation(
                        h_hi[:, jp, pl, :tt], hj, Gelu,
                        bias=lbs[:, j : j + 1], scale=lgs[:, j : j + 1],
                    )
                    nc.scalar.activation(
                        hj, hj, Gelu, bias=lbs[:, j : j + 1], scale=lgs[:, j : j + 1]
                    )
                    nc.vector.tensor_sub(
                        h_lo[:, jp, pl, :tt], hj, h_hi[:, jp, pl, :tt]
                    )

                # ---- previous tile's mm2 on the PE, with this tile's
                # broadcasts at chain 5 and norm/GELU from chain 6 ----
                if prev is not None:
                    emit_mm2(*prev, mid=emit_bc, post=emit_norm_gelu)
                else:
                    backlog.append(emit_bc)
                    backlog.extend(
                        (lambda j=j, f=emit_norm_gelu: f(j, spread=True))
                        for j in range(NJ)
                    )
                prev = (h_hi, h_lo, t0, tt)

            if len(tiles) == 1:  # safety for tiny C: no tile-1 DMA slots
                emit_w2(8, NK)
                nc.sync.dma_start(b2s[:], b2[:])
            for fn in backlog:
                fn()
            emit_mm2(*prev)

    nc.compile()
    return nc


def _route(x64, Wg64, bg64):
    """Host gating: per-token top-2 expert ids and renormalized weights."""
    logits = x64 @ Wg64 + bg64                      # [N, E] fp64
    order = np.argsort(-logits, axis=1, kind="stable")[:, :TOPK]
    l0 = np.take_along_axis(logits, order, axis=1)  # [N, 2] descending
    w0 = 1.0 / (1.0 + np.exp(l0[:, 1] - l0[:, 0]))
    w = np.stack([w0, 1.0 - w0], axis=1)
    return order, w


def _split8(a):
    hi = a.astype(F8)
    lo = (a - hi.astype(np.float32)).astype(F8)
    return hi, lo


def kernel(x, W1, b1, ln_g, ln_b, W2, b2, Wg, bg):
    x = np.ascontiguousarray(np.asarray(x, dtype=np.float32))
    W1 = np.asarray(W1, dtype=np.float32)
    b1 = np.asarray(b1, dtype=np.float32)
    ln_g = np.asarray(ln_g, dtype=np.float32)
    ln_b = np.asarray(ln_b, dtype=np.float32)
    W2 = np.asarray(W2, dtype=np.float32)
    b2 = np.asarray(b2, dtype=np.float32)
    Wg = np.asarray(Wg, dtype=np.float32)
    bg = np.asarray(bg, dtype=np.float32)
    N = x.shape[0]

    order, w = _route(x.astype(np.float64), Wg.astype(np.float64), bg.astype(np.float64))

    tok_idx, tok_w = [], []
    for e in range(E):
        sel = np.nonzero((order[:, 0] == e) | (order[:, 1] == e))[0]
        we = np.where(order[sel, 0] == e, w[sel, 0], w[sel, 1]).astype(np.float32)
        tok_idx.append(sel)
        tok_w.append(we)
    C = max(GRAN, int(-(-max(len(s) for s in tok_idx) // GRAN)) * GRAN)

    if C not in _kernel_cache:
        _kernel_cache[C] = _build(C)
    nc = _kernel_cache[C]

    in_maps = []
    for e in range(E):
        idx = np.zeros(C, dtype=np.int64)
        idx[: len(tok_idx[e])] = tok_idx[e]
        xg = x[idx] * SX                              # [C, D]
        xh, xl = _split8(xg)
        # [C, D] -> [P, DP, 2, C]
        xh_d = np.ascontiguousarray(xh.reshape(C, DP, 2, P).transpose(3, 1, 2, 0))
        xl_d = np.ascontiguousarray(xl.reshape(C, DP, 2, P).transpose(3, 1, 2, 0))
        w1h, w1l = _split8(W1[e] * SW1)               # [D, H]
        w1h_d = np.ascontiguousarray(
            w1h.reshape(DP, 2, P, NJ, P).transpose(2, 3, 0, 1, 4)
        )
        w1l_d = np.ascontiguousarray(
            w1l.reshape(DP, 2, P, NJ, P).transpose(2, 3, 0, 1, 4)
        )
        # S-fold: column-sum of W1 (scaled), replicated over 32 lhsT columns
        w1s = W1[e].sum(axis=1) * SW1S                # [D]
        w1sh, w1sl = _split8(w1s)
        w1sh_d = np.ascontiguousarray(np.broadcast_to(
            w1sh.reshape(DP, 2, P).transpose(2, 0, 1)[:, :, :, None], (P, DP, 2, 32)
        ).astype(F8))
        w1sl_d = np.ascontiguousarray(np.broadcast_to(
            w1sl.reshape(DP, 2, P).transpose(2, 0, 1)[:, :, :, None], (P, DP, 2, 32)
        ).astype(F8))
        sb1h_d = np.full((1, 1), b1[e].sum() / H, dtype=np.float32)
        w2h, w2l = _split8(W2[e] * SW2)               # [H, H]
        w2h_d = np.ascontiguousarray(
            w2h.reshape(KP, 2, P, NK, P).transpose(2, 3, 0, 1, 4)
        )
        w2l_d = np.ascontiguousarray(
            w2l.reshape(KP, 2, P, NK, P).transpose(2, 3, 0, 1, 4)
        )
        in_maps.append(
            {
                "XH": xh_d,
                "XL": xl_d,
                "W1H": w1h_d,
                "W1L": w1l_d,
                "W1SH": w1sh_d,
                "W1SL": w1sl_d,
                "SB1H": sb1h_d,
                "W2H": w2h_d,
                "W2L": w2l_d,
                "b1": np.ascontiguousarray(b1[e].reshape(NJ, P).T),
                "b2": np.ascontiguousarray(b2[e].reshape(NK, P).T),
                "lg": np.ascontiguousarray(ln_g[e].reshape(NJ, P).T),
                "lb": np.ascontiguousarray(ln_b[e].reshape(NJ, P).T),
            }
        )

    results = _run(C, nc, in_maps)

    y = np.zeros((N, H), dtype=np.float32)
    for e in range(E):
        cnt = len(tok_idx[e])
        eoT = results[e]["outT"].reshape(H, C).astype(np.float32)
        y[tok_idx[e]] += tok_w[e][:, None] * eoT[:, :cnt].T
    return y


_neff_cache: dict[int, str] = {}


def _run(C, nc, in_maps):
    if axon_active():
        # PJRT path; NEFF compile is cached by libneuronxla.
        return run_bass_kernel_spmd(nc, in_maps, core_ids=list(range(E))).results
    # Native path: compile once per capacity, then execute the cached NEFF.
    from concourse.bass_utils import compile_bass_kernel, run_neff

    if C not in _neff_cache:
        _neff_cache[C] = compile_bass_kernel(nc, tempfile.mkdtemp())
    out_maps = [{"outT": np.zeros((NK, P, C), dtype=BF)} for _ in range(E)]
    in_maps = [m.copy() for m in in_maps]
    if nc.partition_id_tensor:
        for core_id, m in enumerate(in_maps):
            m[nc.partition_id_tensor.name] = np.array([[core_id]], dtype=np.uint32)
    return run_neff(
        _neff_cache[C],
        in_maps,
        out_maps,
        core_ids=list(range(E)),
        has_collectives=False,
    )



# revision 3
# speedup vs baseline: 1.1622x; 1.1622x over previous
# MoE (top-2 of 8 experts) kernel for 8 Trainium2 NeuronCores.
#
# Strategy: expert-parallel sparse routing with fp8 DoubleRow matmuls and
# GRADED PRECISION. Host computes the gating network and per-expert token
# lists; core e runs expert e's FFN (x@W1+b1 -> LayerNorm -> erf-GELU ->
# @W2+b2) on its routed tokens, sorted by combine weight (descending).
# Matmuls run as fp8(e4m3) DoubleRow with hi/lo error compensation, but the
# compensation passes cover only a column prefix of each (w-sorted) token
# tile: tokens with large combine weights get full 3-pass accuracy, tokens
# with small weights (and padding) get 1-2 passes. The per-slot pass plan is
# chosen by a Lagrangian knapsack on the w^2 profile so the end-to-end
# rel err stays under a fixed budget while minimizing PE cycles.
# All weights live in SBUF (10MB fp8), loaded once in contiguous per-chunk
# DMAs. LayerNorm S-sums come from a 1-pass W1-column-sum lhsT row; Q-sums
# use a paired-fp8 DoubleRow ones-matmul on h^2; per-token stats broadcast
# across partitions with K=1 f32r matmuls, emitted mid-mm2 so the PE never
# waits on the stats chain. The first tile's normalize/GELU backlog is
# drip-fed through the second tile's mm1 loop.

import tempfile

import ml_dtypes
import numpy as np

import concourse.bacc as bacc
import concourse.mybir as mybir
import concourse.tile as tile
from concourse._compat import axon_active
from concourse.bass_utils import run_bass_kernel_spmd

P = 128
D, H, E, TOPK = 1024, 2048, 8, 2
DP, KP, NJ, NK = D // 256, H // 256, H // P, H // P  # 4, 8, 16, 16
LN_EPS = 1e-5
TT = 512           # main token tile
GRAN = 16          # capacity granularity
SX, SW1, SW2 = 16.0, 256.0, 256.0   # fp8 pre-quantization scales
SW1S = 32.0        # scale for the W1 column-sum row (S-fold)
F8 = ml_dtypes.float8_e4m3
BF = ml_dtypes.bfloat16

# Error model for the pass planner (err^2 contributions, measured on the
# reference input distribution; used as a heuristic for any input).
ERR_TARGET = 1.55e-2
E1_2, E1_1 = 5.98e-4, 13.49e-4   # mm1 at 2 passes / 1 pass
E2_2, E2_1 = 7.03e-4, 14.40e-4   # mm2 at 2 passes / 1 pass
FLOOR2 = 0.23e-4
C1_NS = 16 * 4 * 0.5 / 2.4       # PE ns/slot for one extra mm1 pass
C2_NS = 16 * 8 * 0.5 / 2.4

_kernel_cache: dict[tuple, object] = {}


def _t_tiles(C):
    tiles, t0 = [], 0
    while t0 < C:
        tt = TT if C - t0 >= TT else C - t0
        tiles.append((t0, tt))
        t0 += tt
    # Tail tile last: its cheap mm2 is the only un-overlapped one, and
    # full-size norm/GELU phases pair with full-size mm2 phases.
    return tiles


def _plan(u):
    """Per-slot pass levels (p, q) for mm1/mm2 given the normalized w^2 slot
    profile u, then snapped to per-tile compensation widths."""
    C = len(u)

    def plan_at(lam):
        p = np.full(C, 3, dtype=np.int64)
        q = np.full(C, 3, dtype=np.int64)
        p[u <= C1_NS / (lam * E1_2)] = 2
        p[(p == 2) & (u <= C1_NS / (lam * (E1_1 - E1_2)))] = 1
        q[u <= C2_NS / (lam * E2_2)] = 2
        q[(q == 2) & (u <= C2_NS / (lam * (E2_1 - E2_2)))] = 1
        return p, q

    def err2(p, q):
        t = np.zeros(C)
        t[p == 2] += E1_2
        t[p == 1] += E1_1
        t[q == 2] += E2_2
        t[q == 1] += E2_1
        return float((u * t).sum())

    B = ERR_TARGET * ERR_TARGET - FLOOR2
    lo, hi = 1e0, 1e14
    for _ in range(100):
        lam = (lo * hi) ** 0.5
        p, q = plan_at(lam)
        if err2(p, q) > B:
            lo = lam   # too much error -> fewer reductions needed
        else:
            hi = lam
    p, q = plan_at(hi)

    tiles = []
    for (t0, tt) in _t_tiles(C):
        def width(lev, need):
            n = int((lev[t0 : t0 + tt] >= need).sum())
            if n == 0:
                return 0
            n = min(tt, (n + 31) // 32 * 32)
            return max(n, min(tt, 128))
        g2, g3 = width(p, 2), width(p, 3)
        k2, k3 = width(q, 2), width(q, 3)
        tiles.append((tt, g2, g3, k2, k3))
    return tuple(tiles)


def _build(C: int, plan):
    f32, f32r, bf16, f8 = (
        mybir.dt.float32, mybir.dt.float32r, mybir.dt.bfloat16, mybir.dt.float8e4
    )
    DR = mybir.MatmulPerfMode.DoubleRow
    Mul, Add = mybir.AluOpType.mult, mybir.AluOpType.add
    nc = bacc.Bacc("TRN2", target_bir_lowering=False, debug=False, num_devices=8)
    XH = nc.dram_tensor("XH", [P, DP, 2, C], f8, kind="ExternalInput").ap()
    XL = nc.dram_tensor("XL", [P, DP, 2, C], f8, kind="ExternalInput").ap()
    W1H = nc.dram_tensor("W1H", [P, NJ, DP, 2, P], f8, kind="ExternalInput").ap()
    W1L = nc.dram_tensor("W1L", [P, NJ, DP, 2, P], f8, kind="ExternalInput").ap()
    W1SH = nc.dram_tensor("W1SH", [P, DP, 2, 32], f8, kind="ExternalInput").ap()
    SB1H = nc.dram_tensor("SB1H", [1, 1], f32, kind="ExternalInput").ap()
    W2H = nc.dram_tensor("W2H", [P, NK, KP, 2, P], f8, kind="ExternalInput").ap()
    W2L = nc.dram_tensor("W2L", [P, NK, KP, 2, P], f8, kind="ExternalInput").ap()
    YA = nc.dram_tensor("YA", [1, 1], f32, kind="ExternalInput").ap()
    YB = nc.dram_tensor("YB", [1, 1], f32, kind="ExternalInput").ap()
    b1 = nc.dram_tensor("b1", [P, NJ], f32, kind="ExternalInput").ap()
    b2 = nc.dram_tensor("b2", [P, NK], f32, kind="ExternalInput").ap()
    lg = nc.dram_tensor("lg", [P, NJ], f32, kind="ExternalInput").ap()
    lb = nc.dram_tensor("lb", [P, NJ], f32, kind="ExternalInput").ap()
    outT = nc.dram_tensor("outT", [P, NK, C], bf16, kind="ExternalOutput").ap()

    Gelu = mybir.ActivationFunctionType.Gelu
    Sqrt = mybir.ActivationFunctionType.Sqrt
    Ident = mybir.ActivationFunctionType.Identity

    with tile.TileContext(nc) as tc:
        with (
            tc.tile_pool(name="const", bufs=1) as constp,
            tc.tile_pool(name="wp", bufs=1) as wp,
            tc.tile_pool(name="xp", bufs=1) as xp,
            tc.tile_pool(name="hp", bufs=2) as hp,
            tc.tile_pool(name="hxp", bufs=2) as hxp,
            tc.tile_pool(name="sqp", bufs=2) as sqp,
            tc.tile_pool(name="op", bufs=2) as op,
            tc.tile_pool(name="statp", bufs=1) as statp,
            tc.tile_pool(name="ps_mm", bufs=6, space="PSUM") as ps_mm,
            tc.tile_pool(name="ps_acc", bufs=1, space="PSUM") as ps_acc,
        ):
            b1s = constp.tile([P, NJ], f32)
            b2s = constp.tile([P, NK], f32)
            lgs = constp.tile([P, NJ], f32)
            lbs = constp.tile([P, NJ], f32)
            w1sh = constp.tile([P, DP, 2, 32], f8)
            sb1h = constp.tile([1, 1], f32)
            ya = constp.tile([1, 1], f32)
            yb = constp.tile([1, 1], f32)

            def emit_const_dmas():
                nc.sync.dma_start(b1s[:], b1[:])
                nc.sync.dma_start(lgs[:], lg[:])
                nc.sync.dma_start(lbs[:], lb[:])
                nc.sync.dma_start(w1sh[:], W1SH[:])
                nc.sync.dma_start(sb1h[:], SB1H[:])
                nc.sync.dma_start(ya[:], YA[:])
                nc.sync.dma_start(yb[:], YB[:])

            ones_q = constp.tile([P, 2, 32], f8)   # lhsT for Q paired DR sums
            nc.any.memset(ones_q[:], 1.0)
            eps_t = constp.tile([1, 1], f32)
            nc.any.memset(eps_t[:], LN_EPS)

            # fp8 weights, SBUF-resident for the whole kernel, streamed in
            # contiguous per-chunk DMAs staged around the first two tiles.
            w1h = wp.tile([P, NJ, DP, 2, P], f8)
            w1l = wp.tile([P, NJ, DP, 2, P], f8)
            w2h = wp.tile([P, NK, KP, 2, P], f8)
            w2l = wp.tile([P, NK, KP, 2, P], f8)

            def emit_w1(a, b):
                nc.sync.dma_start(w1h[:, a:b], W1H[:, a:b])
                nc.sync.dma_start(w1l[:, a:b], W1L[:, a:b])

            def emit_w2(a, b):
                nc.sync.dma_start(w2h[:, a:b], W2H[:, a:b])
                nc.sync.dma_start(w2l[:, a:b], W2L[:, a:b])

            def emit_mm2(h_hi, h_lo, t0, tt, k2, k3, mid=None, post=None):
                # Graded fp8 mm2: full (w2h,h_hi) pass + (w2l,h_hi) over
                # [:k2] + (w2h,h_lo) over [:k3]; b2 added at the evict.
                # mid() runs after chain 5 (the next tile's stats-broadcast
                # matmuls); post(j) runs twice per chain from chain 6 (the
                # next tile's normalize/GELU work). With no post work (final
                # tile) the evict alternates DVE/ACT so neither throttles the
                # short chains.
                step = 0
                passes = [(w2h, h_hi, tt)]
                if k2:
                    passes.append((w2l, h_hi, k2))
                if k3:
                    passes.append((w2h, h_lo, k3))
                npass = len(passes)
                ot = None
                for k in range(NK):
                    pm = ps_mm.tile([P, TT], f32, tag="mm", name="mm2")[:, :tt]
                    for pi, (wt, ht, g) in enumerate(passes):
                        for kp in range(KP):
                            nc.tensor.matmul(
                                pm[:, :g],
                                wt[:, k, kp, :, :],
                                ht[:, kp, :, :g],
                                start=(pi == 0 and kp == 0),
                                stop=(pi == npass - 1 and kp == KP - 1),
                                perf_mode=DR,
                            )
                    if k % 2 == 0:
                        # batch 2 output chunks per DMA: halves the number of
                        # HWDGE acquisitions (the exclusive HWDGE device
                        # serializes the kernel drain otherwise)
                        ot = op.tile([P, 2, tt], bf16, tag=f"out{tt}",
                                     name="out", bufs=(2 if tt == TT else 5))
                    nc.vector.tensor_scalar(
                        ot[:, k % 2, :], pm[:], 1.0 / SW2, b2s[:, k : k + 1],
                        Mul, Add,
                    )
                    if k % 2 == 1:
                        nc.sync.dma_start(
                            outT[:, k - 1 : k + 1, t0 : t0 + tt], ot[:]
                        )
                    if k == 5 and mid is not None:
                        mid()
                    if k >= 6 and post is not None:
                        for _ in range(2):
                            if step < NJ:
                                post(step)
                                step += 1
                while post is not None and step < NJ:
                    post(step)
                    step += 1

            prev = None
            backlog = []
            tiles = _t_tiles(C)

            def emit_x(i):
                # Prefetch tile i's activations (one tile ahead of use) so
                # the DMA never queues behind an out-DMA whose SEQ wait only
                # clears at the end of an mm2 phase.
                t0, tt = tiles[i]
                g2 = plan[i][1]
                xh = xp.tile([P, DP, 2, TT], f8, tag="xh", name="xh", bufs=2)
                nc.sync.dma_start(xh[:, :, :, :tt], XH[:, :, :, t0 : t0 + tt])
                if g2:
                    xl = xp.tile([P, DP, 2, TT], f8, tag="xl", name="xl", bufs=2)
                    nc.sync.dma_start(xl[:, :, :, :g2], XL[:, :, :, t0 : t0 + g2])
                else:
                    xl = None
                return xh, xl

            x_pref = None
            for tile_i, (t0, tt) in enumerate(tiles):
                tt_, g2, g3, k2, k3 = plan[tile_i]
                assert tt_ == tt
                if tile_i == 0:
                    emit_w1(0, 1)  # W1 j=0 ahead of x so the first chain starts fast
                    x_pref = emit_x(0)
                xh, xl = x_pref
                if tile_i == 0:
                    emit_w1(1, 3)
                    emit_const_dmas()
                h = hp.tile(
                    [P, NJ, tt], bf16, tag=f"h{tt}", name="h",
                    bufs=(2 if tt == TT else 1),
                )
                h_hi = hxp.tile(
                    [P, KP, 2, tt], f8, tag=f"hh{tt}", name="h_hi",
                    bufs=(2 if tt == TT else 1),
                )
                h_lo = hxp.tile(
                    [P, KP, 2, tt], f8, tag=f"hl{tt}", name="h_lo",
                    bufs=(2 if tt == TT else 1),
                ) if k3 else None
                s_ps = ps_acc.tile([32, TT], f32, tag="sacc", name="sacc")[:, :tt]
                q_ps = ps_acc.tile([32, TT], f32, tag="qacc", name="qacc")[:, :tt]

                # ---- graded mm1; Q ones-matmuls deferred one pair so the PE
                # never waits on the ACT evict / DVE square chain; tile0's
                # norm/GELU backlog drip-fed through tile1's loop ----
                m1_passes = [(w1h, xh, tt)]
                if g2:
                    m1_passes.append((w1h, xl, g2))
                if g3:
                    m1_passes.append((w1l, xh, g3))
                np1 = len(m1_passes)
                pend_q = None
                sq = None
                for j in range(NJ):
                    if tile_i == 0:
                        if j == 0:
                            emit_w1(3, 8)
                        elif j == 4:
                            emit_w1(8, NJ)
                        elif j == 8:
                            emit_w2(0, 4)
                        elif j == 12:
                            emit_w2(4, 8)
                    elif tile_i == 1:
                        if j == 0:
                            emit_w2(8, 12)
                        elif j == 4:
                            emit_w2(12, NK)
                        elif j == 8:
                            nc.sync.dma_start(b2s[:], b2[:])
                    pm = ps_mm.tile([P, TT], f32, tag="mm", name="mm1")[:, :tt]
                    for pi, (wt, xt, g) in enumerate(m1_passes):
                        for dp in range(DP):
                            nc.tensor.matmul(
                                pm[:, :g],
                                wt[:, j, dp, :, :],
                                xt[:, dp, :, :g],
                                start=(pi == 0 and dp == 0),
                                stop=(pi == np1 - 1 and dp == DP - 1),
                                perf_mode=DR,
                            )
                    nc.scalar.activation(
                        h[:, j, :], pm[:], Ident,
                        bias=b1s[:, j : j + 1], scale=1.0 / (SX * SW1),
                    )
                    if j % 2 == 0:
                        sq = sqp.tile([P, 2, TT], f8, tag="sq", name="sq")
                    nc.vector.tensor_mul(sq[:, j % 2, :tt], h[:, j, :], h[:, j, :])
                    if j % 2 == 1:
                        if pend_q is not None:
                            jp, sqt = pend_q
                            nc.tensor.matmul(
                                q_ps[:], ones_q[:], sqt[:, :, :tt],
                                start=(jp == 0), stop=(jp == NJ // 2 - 1),
                                perf_mode=DR,
                            )
                        pend_q = (j // 2, sq)
                    if backlog:
                        backlog.pop(0)()
                while backlog:  # finish tile0's backlog before mm2(0) reads h
                    backlog.pop(0)()
                # S-fold: the W1-column-sum row, 1-pass DR into s_ps
                for dp in range(DP):
                    nc.tensor.matmul(
                        s_ps[:],
                        w1sh[:, dp, :, :],
                        xh[:, dp, :, :tt],
                        start=(dp == 0),
                        stop=(dp == DP - 1),
                        perf_mode=DR,
                    )
                jp, sqt = pend_q
                nc.tensor.matmul(
                    q_ps[:], ones_q[:], sqt[:, :, :tt],
                    start=(jp == 0), stop=(jp == NJ // 2 - 1),
                    perf_mode=DR,
                )

                # ---- LN stats (DVE/ACT only; broadcasts happen mid-mm2) ----
                mu = statp.tile([1, TT], f32, tag="mu", name="mu")[:, :tt]
                nc.vector.tensor_scalar(
                    mu[:], s_ps[0:1, :], 1.0 / (SX * SW1S * H), sb1h[:], Mul, Add
                )
                tmp = statp.tile([1, TT], f32, tag="tmp", name="tmp")[:, :tt]
                nc.vector.tensor_scalar_mul(tmp[:], q_ps[0:1, :], 1.0 / H)
                tmp2 = statp.tile([1, TT], f32, tag="tmp2", name="tmp2")[:, :tt]
                nc.vector.tensor_mul(tmp2[:], mu[:], mu[:])
                nc.vector.tensor_sub(tmp[:], tmp[:], tmp2[:])          # var
                nc.vector.tensor_scalar(tmp2[:], tmp[:], ya[:], yb[:], Mul, Add)
                nc.vector.tensor_mul(tmp[:], tmp[:], tmp2[:])
                nc.vector.tensor_mul(tmp[:], tmp[:], tmp2[:])
                nc.vector.tensor_scalar(tmp[:], tmp[:], -0.5, 1.5, Mul, Add)
                nc.vector.tensor_mul(tmp[:], tmp[:], tmp2[:])          # rstd
                a_row = statp.tile([1, TT], bf16, tag="a_row", name="a_row", bufs=2)
                nc.vector.tensor_copy(a_row[:, :tt], tmp[:])
                b_row = statp.tile([1, TT], bf16, tag="b_row", name="b_row", bufs=2)
                nc.vector.tensor_mul(b_row[:, :tt], mu[:], tmp[:])

                a_sb = statp.tile([P, TT], bf16, tag="a_sb", name="a_sb", bufs=2)
                b_sb = statp.tile([P, TT], bf16, tag="b_sb", name="b_sb", bufs=2)

                def emit_bc(a_row=a_row, b_row=b_row, a_sb=a_sb, b_sb=b_sb, tt=tt):
                    # per-token stat rows -> all partitions, on the idle
                    # GPSIMD engine (frees the PE matmuls, the DVE
                    # PSUM-copies, and two PSUM banks)
                    nc.gpsimd.partition_broadcast(a_sb[:, :tt], a_row[:, :tt])
                    nc.gpsimd.partition_broadcast(b_sb[:, :tt], b_row[:, :tt])

                def emit_norm_gelu(
                    j, h=h, h_hi=h_hi, h_lo=h_lo, a_sb=a_sb, b_sb=b_sb, tt=tt,
                    k3=k3, spread=False,
                ):
                    # normalize (DVE bf16) + GELU (ACT: fp8 h_hi full width;
                    # bf16 + h_lo residual only over [:k3]). In spread mode
                    # (tile0 backlog, no mm2 window to hide in) the h_lo path
                    # uses a Pool copy and alternating Pool/DVE subs so the
                    # work balances across all three engines.
                    jp, pl = j // 2, j % 2
                    hj = h[:, j, :tt]
                    nc.vector.tensor_mul(hj, hj, a_sb[:, :tt])
                    nc.vector.tensor_sub(hj, hj, b_sb[:, :tt])
                    if spread:
                        nc.scalar.activation(
                            hj, hj, Gelu,
                            bias=lbs[:, j : j + 1], scale=lgs[:, j : j + 1],
                        )
                        nc.gpsimd.tensor_copy(h_hi[:, jp, pl, :tt], hj)
                        if k3:
                            eng = nc.gpsimd if j % 2 else nc.vector
                            eng.tensor_sub(
                                h_lo[:, jp, pl, :k3], hj[:, :k3], h_hi[:, jp, pl, :k3]
                            )
                        return
                    nc.scalar.activation(
                        h_hi[:, jp, pl, :tt], hj, Gelu,
                        bias=lbs[:, j : j + 1], scale=lgs[:, j : j + 1],
                    )
                    if k3:
                        nc.scalar.activation(
                            hj[:, :k3], hj[:, :k3], Gelu,
                            bias=lbs[:, j : j + 1], scale=lgs[:, j : j + 1],
                        )
                        nc.vector.tensor_sub(
                            h_lo[:, jp, pl, :k3], hj[:, :k3], h_hi[:, jp, pl, :k3]
                        )

                if tile_i + 1 < len(tiles):
                    x_pref = emit_x(tile_i + 1)

                # ---- previous tile's mm2 on the PE, with this tile's
                # broadcasts at chain 5 and norm/GELU from chain 6 ----
                if prev is not None:
                    emit_mm2(*prev, mid=emit_bc, post=emit_norm_gelu)
                else:
                    backlog.append(emit_bc)
                    backlog.extend(
                        (lambda j=j, f=emit_norm_gelu: f(j, spread=True))
                        for j in range(NJ)
                    )
                prev = (h_hi, h_lo, t0, tt, k2, k3)

            if len(tiles) == 1:  # safety for tiny C: no tile-1 DMA slots
                emit_w2(8, NK)
                nc.sync.dma_start(b2s[:], b2[:])
            for fn in backlog:
                fn()
            emit_mm2(*prev)

    nc.compile()
    return nc


def _route(x64, Wg64, bg64):
    """Host gating: per-token top-2 expert ids and renormalized weights."""
    logits = x64 @ Wg64 + bg64                      # [N, E] fp64
    order = np.argsort(-logits, axis=1, kind="stable")[:, :TOPK]
    l0 = np.take_along_axis(logits, order, axis=1)  # [N, 2] descending
    w0 = 1.0 / (1.0 + np.exp(l0[:, 1] - l0[:, 0]))
    w = np.stack([w0, 1.0 - w0], axis=1)
    return order, w


def _split8(a):
    hi = a.astype(F8)
    lo = (a - hi.astype(np.float32)).astype(F8)
    return hi, lo


def kernel(x, W1, b1, ln_g, ln_b, W2, b2, Wg, bg):
    x = np.ascontiguousarray(np.asarray(x, dtype=np.float32))
    W1 = np.asarray(W1, dtype=np.float32)
    b1 = np.asarray(b1, dtype=np.float32)
    ln_g = np.asarray(ln_g, dtype=np.float32)
    ln_b = np.asarray(ln_b, dtype=np.float32)
    W2 = np.asarray(W2, dtype=np.float32)
    b2 = np.asarray(b2, dtype=np.float32)
    Wg = np.asarray(Wg, dtype=np.float32)
    bg = np.asarray(bg, dtype=np.float32)
    N = x.shape[0]

    order, w = _route(x.astype(np.float64), Wg.astype(np.float64), bg.astype(np.float64))

    tok_idx, tok_w = [], []
    for e in range(E):
        sel = np.nonzero((order[:, 0] == e) | (order[:, 1] == e))[0]
        we = np.where(order[sel, 0] == e, w[sel, 0], w[sel, 1]).astype(np.float32)
        o = np.argsort(-we, kind="stable")   # high-combine-weight slots first
        tok_idx.append(sel[o])
        tok_w.append(we[o])
    C = max(GRAN, int(-(-max(len(s) for s in tok_idx) // GRAN)) * GRAN)

    # normalized w^2 slot profile -> graded pass plan (shared by all cores)
    u = np.zeros(C)
    for e in range(E):
        u[: len(tok_w[e])] += tok_w[e].astype(np.float64) ** 2
    u /= u.sum()
    plan = _plan(u)

    key = (C, plan)
    if key not in _kernel_cache:
        _kernel_cache[key] = _build(C, plan)
    nc = _kernel_cache[key]

    in_maps = []
    for e in range(E):
        idx = np.zeros(C, dtype=np.int64)
        idx[: len(tok_idx[e])] = tok_idx[e]
        xg = x[idx] * SX                              # [C, D]
        xg[len(tok_idx[e]):] = 0.0
        xh, xl = _split8(xg)
        # [C, D] -> [P, DP, 2, C]
        xh_d = np.ascontiguousarray(xh.reshape(C, DP, 2, P).transpose(3, 1, 2, 0))
        xl_d = np.ascontiguousarray(xl.reshape(C, DP, 2, P).transpose(3, 1, 2, 0))
        w1h, w1l = _split8(W1[e] * SW1)               # [D, H]
        w1h_d = np.ascontiguousarray(
            w1h.reshape(DP, 2, P, NJ, P).transpose(2, 3, 0, 1, 4)
        )
        w1l_d = np.ascontiguousarray(
            w1l.reshape(DP, 2, P, NJ, P).transpose(2, 3, 0, 1, 4)
        )
        # S-fold: column-sum of W1 (scaled), replicated over 32 lhsT columns
        w1s = W1[e].sum(axis=1) * SW1S                # [D]
        w1sh, _ = _split8(w1s)
        w1sh_d = np.ascontiguousarray(np.broadcast_to(
            w1sh.reshape(DP, 2, P).transpose(2, 0, 1)[:, :, :, None], (P, DP, 2, 32)
        ).astype(F8))
        sb1h_d = np.full((1, 1), b1[e].sum() / H, dtype=np.float32)
        w1c = W1[e] - W1[e].mean(axis=1, keepdims=True)
        vbar = float((w1c * w1c).sum() / H + np.var(b1[e]))
        y0 = 1.0 / np.sqrt(vbar + LN_EPS)
        ya_d = np.full((1, 1), -0.5 * y0 ** 3, dtype=np.float32)
        yb_d = np.full((1, 1), 1.5 * y0, dtype=np.float32)
        w2h, w2l = _split8(W2[e] * SW2)               # [H, H]
        w2h_d = np.ascontiguousarray(
            w2h.reshape(KP, 2, P, NK, P).transpose(2, 3, 0, 1, 4)
        )
        w2l_d = np.ascontiguousarray(
            w2l.reshape(KP, 2, P, NK, P).transpose(2, 3, 0, 1, 4)
        )
        in_maps.append(
            {
                "XH": xh_d,
                "XL": xl_d,
                "W1H": w1h_d,
                "W1L": w1l_d,
                "W1SH": w1sh_d,
                "SB1H": sb1h_d,
                "YA": ya_d,
                "YB": yb_d,
                "W2H": w2h_d,
                "W2L": w2l_d,
                "b1": np.ascontiguousarray(b1[e].reshape(NJ, P).T),
                "b2": np.ascontiguousarray(b2[e].reshape(NK, P).T),
                "lg": np.ascontiguousarray(ln_g[e].reshape(NJ, P).T),
                "lb": np.ascontiguousarray(ln_b[e].reshape(NJ, P).T),
            }
        )

    results = _run(key, nc, in_maps)

    y = np.zeros((N, H), dtype=np.float32)
    for e in range(E):
        cnt = len(tok_idx[e])
        eoT = (
            results[e]["outT"].transpose(1, 0, 2).reshape(H, C).astype(np.float32)
        )
        y[tok_idx[e]] += tok_w[e][:, None] * eoT[:, :cnt].T
    return y


_neff_cache: dict[tuple, str] = {}


def _run(key, nc, in_maps):
    C = key[0]
    if axon_active():
        # PJRT path; NEFF compile is cached by libneuronxla.
        return run_bass_kernel_spmd(nc, in_maps, core_ids=list(range(E))).results
    # Native path: compile once per capacity, then execute the cached NEFF.
    from concourse.bass_utils import compile_bass_kernel, run_neff

    if key not in _neff_cache:
        _neff_cache[key] = compile_bass_kernel(nc, tempfile.mkdtemp())
    out_maps = [{"outT": np.zeros((P, NK, C), dtype=BF)} for _ in range(E)]
    in_maps = [m.copy() for m in in_maps]
    if nc.partition_id_tensor:
        for core_id, m in enumerate(in_maps):
            m[nc.partition_id_tensor.name] = np.array([[core_id]], dtype=np.uint32)
    return run_neff(
        _neff_cache[key],
        in_maps,
        out_maps,
        core_ids=list(range(E)),
        has_collectives=False,
    )


# revision 4
# speedup vs baseline: 1.1697x; 1.0064x over previous
# MoE (top-2 of 8 experts) kernel for 8 Trainium2 NeuronCores.
#
# Strategy: expert-parallel sparse routing with fp8 DoubleRow matmuls and
# GRADED PRECISION. Host computes the gating network and per-expert token
# lists; core e runs expert e's FFN (x@W1+b1 -> LayerNorm -> erf-GELU ->
# @W2+b2) on its routed tokens, sorted by combine weight (descending).
# Matmuls run as fp8(e4m3) DoubleRow with hi/lo error compensation, but the
# compensation passes cover only a column prefix of each (w-sorted) token
# tile: tokens with large combine weights get full 3-pass accuracy, tokens
# with small weights (and padding) get 1-2 passes. The per-slot pass plan is
# chosen by a Lagrangian knapsack on the w^2 profile so the end-to-end
# rel err stays under a fixed budget while minimizing PE cycles.
# All weights live in SBUF (10MB fp8), loaded once in contiguous per-chunk
# DMAs. LayerNorm S-sums come from a 1-pass W1-column-sum lhsT row; Q-sums
# use a paired-fp8 DoubleRow ones-matmul on h^2; per-token stats broadcast
# across partitions with K=1 f32r matmuls, emitted mid-mm2 so the PE never
# waits on the stats chain. The first tile's normalize/GELU backlog is
# drip-fed through the second tile's mm1 loop.

import tempfile

import ml_dtypes
import numpy as np

import concourse.bacc as bacc
import concourse.mybir as mybir
import concourse.tile as tile
from concourse._compat import axon_active
from concourse.bass_utils import run_bass_kernel_spmd

P = 128
D, H, E, TOPK = 1024, 2048, 8, 2
DP, KP, NJ, NK = D // 256, H // 256, H // P, H // P  # 4, 8, 16, 16
LN_EPS = 1e-5
TT = 512           # main token tile
GRAN = 16          # capacity granularity
SX, SW1, SW2 = 16.0, 256.0, 256.0   # fp8 pre-quantization scales
SW1S = 32.0        # scale for the W1 column-sum row (S-fold)
F8 = ml_dtypes.float8_e4m3
BF = ml_dtypes.bfloat16

# Error model for the pass planner (err^2 contributions, measured on the
# reference input distribution; used as a heuristic for any input).
ERR_TARGET = 1.63e-2
E1_2, E1_1 = 5.98e-4, 13.49e-4   # mm1 at 2 passes / 1 pass
E2_2, E2_1 = 7.03e-4, 14.40e-4   # mm2 at 2 passes / 1 pass
FLOOR2 = 0.23e-4
C1_NS = 16 * 4 * 0.5 / 2.4       # PE ns/slot for one extra mm1 pass
C2_NS = 16 * 8 * 0.5 / 2.4

_kernel_cache: dict[tuple, object] = {}


def _t_tiles(C):
    tiles, t0 = [], 0
    while t0 < C:
        tt = TT if C - t0 >= TT else C - t0
        tiles.append((t0, tt))
        t0 += tt
    # Tail tile last: its cheap mm2 is the only un-overlapped one, and
    # full-size norm/GELU phases pair with full-size mm2 phases.
    return tiles


def _plan(u):
    """Per-slot pass levels (p, q) for mm1/mm2 given the normalized w^2 slot
    profile u, then snapped to per-tile compensation widths."""
    C = len(u)

    def plan_at(lam):
        p = np.full(C, 3, dtype=np.int64)
        q = np.full(C, 3, dtype=np.int64)
        p[u <= C1_NS / (lam * E1_2)] = 2
        p[(p == 2) & (u <= C1_NS / (lam * (E1_1 - E1_2)))] = 1
        q[u <= C2_NS / (lam * E2_2)] = 2
        q[(q == 2) & (u <= C2_NS / (lam * (E2_1 - E2_2)))] = 1
        return p, q

    def err2(p, q):
        t = np.zeros(C)
        t[p == 2] += E1_2
        t[p == 1] += E1_1
        t[q == 2] += E2_2
        t[q == 1] += E2_1
        return float((u * t).sum())

    B = ERR_TARGET * ERR_TARGET - FLOOR2
    lo, hi = 1e0, 1e14
    for _ in range(100):
        lam = (lo * hi) ** 0.5
        p, q = plan_at(lam)
        if err2(p, q) > B:
            lo = lam   # too much error -> fewer reductions needed
        else:
            hi = lam
    p, q = plan_at(hi)

    tiles = []
    for (t0, tt) in _t_tiles(C):
        def width(lev, need):
            n = int((lev[t0 : t0 + tt] >= need).sum())
            if n == 0:
                return 0
            n = min(tt, (n + 31) // 32 * 32)
            return max(n, min(tt, 128))
        g2, g3 = width(p, 2), width(p, 3)
        k2, k3 = width(q, 2), width(q, 3)
        tiles.append((tt, g2, g3, k2, k3))
    return tuple(tiles)


def _build(C: int, plan):
    f32, f32r, bf16, f8 = (
        mybir.dt.float32, mybir.dt.float32r, mybir.dt.bfloat16, mybir.dt.float8e4
    )
    DR = mybir.MatmulPerfMode.DoubleRow
    Mul, Add = mybir.AluOpType.mult, mybir.AluOpType.add
    nc = bacc.Bacc("TRN2", target_bir_lowering=False, debug=False, num_devices=8)
    XH = nc.dram_tensor("XH", [P, DP, 2, C], f8, kind="ExternalInput").ap()
    XL = nc.dram_tensor("XL", [P, DP, 2, C], f8, kind="ExternalInput").ap()
    W1H = nc.dram_tensor("W1H", [P, NJ, DP, 2, P], f8, kind="ExternalInput").ap()
    W1L = nc.dram_tensor("W1L", [P, NJ, DP, 2, P], f8, kind="ExternalInput").ap()
    W1SH = nc.dram_tensor("W1SH", [P, DP, 2, 32], f8, kind="ExternalInput").ap()
    SB1H = nc.dram_tensor("SB1H", [1, 1], f32, kind="ExternalInput").ap()
    W2H = nc.dram_tensor("W2H", [P, NK, KP, 2, P], f8, kind="ExternalInput").ap()
    W2L = nc.dram_tensor("W2L", [P, NK, KP, 2, P], f8, kind="ExternalInput").ap()
    YA = nc.dram_tensor("YA", [1, 1], f32, kind="ExternalInput").ap()
    YB = nc.dram_tensor("YB", [1, 1], f32, kind="ExternalInput").ap()
    b1 = nc.dram_tensor("b1", [P, NJ], f32, kind="ExternalInput").ap()
    b2 = nc.dram_tensor("b2", [P, NK], f32, kind="ExternalInput").ap()
    lg = nc.dram_tensor("lg", [P, NJ], f32, kind="ExternalInput").ap()
    lb = nc.dram_tensor("lb", [P, NJ], f32, kind="ExternalInput").ap()
    outT = nc.dram_tensor("outT", [P, NK, C], bf16, kind="ExternalOutput").ap()

    Gelu = mybir.ActivationFunctionType.Gelu
    Sqrt = mybir.ActivationFunctionType.Sqrt
    Ident = mybir.ActivationFunctionType.Identity

    with tile.TileContext(nc) as tc:
        with (
            tc.tile_pool(name="const", bufs=1) as constp,
            tc.tile_pool(name="wp", bufs=1) as wp,
            tc.tile_pool(name="xp", bufs=1) as xp,
            tc.tile_pool(name="hp", bufs=2) as hp,
            tc.tile_pool(name="hxp", bufs=2) as hxp,
            tc.tile_pool(name="sqp", bufs=2) as sqp,
            tc.tile_pool(name="op", bufs=2) as op,
            tc.tile_pool(name="statp", bufs=1) as statp,
            tc.tile_pool(name="ps_mm", bufs=6, space="PSUM") as ps_mm,
            tc.tile_pool(name="ps_acc", bufs=1, space="PSUM") as ps_acc,
        ):
            b1s = constp.tile([P, NJ], f32)
            b2s = constp.tile([P, NK], f32)
            lgs = constp.tile([P, NJ], f32)
            lbs = constp.tile([P, NJ], f32)
            w1sh = constp.tile([P, DP, 2, 32], f8)
            sb1h = constp.tile([1, 1], f32)
            ya = constp.tile([1, 1], f32)
            yb = constp.tile([1, 1], f32)

            def emit_const_dmas():
                nc.sync.dma_start(b1s[:], b1[:])
                nc.sync.dma_start(lgs[:], lg[:])
                nc.sync.dma_start(lbs[:], lb[:])
                nc.sync.dma_start(w1sh[:], W1SH[:])
                nc.sync.dma_start(sb1h[:], SB1H[:])
                nc.sync.dma_start(ya[:], YA[:])
                nc.sync.dma_start(yb[:], YB[:])

            ones_q = constp.tile([P, 2, 32], f8)   # lhsT for Q paired DR sums
            nc.any.memset(ones_q[:], 1.0)
            eps_t = constp.tile([1, 1], f32)
            nc.any.memset(eps_t[:], LN_EPS)

            # fp8 weights, SBUF-resident for the whole kernel, streamed in
            # contiguous per-chunk DMAs staged around the first two tiles.
            w1h = wp.tile([P, NJ, DP, 2, P], f8)
            w1l = wp.tile([P, NJ, DP, 2, P], f8)
            w2h = wp.tile([P, NK, KP, 2, P], f8)
            w2l = wp.tile([P, NK, KP, 2, P], f8)

            def emit_w1(a, b):
                nc.sync.dma_start(w1h[:, a:b], W1H[:, a:b])
                nc.sync.dma_start(w1l[:, a:b], W1L[:, a:b])

            def emit_w2(a, b):
                nc.sync.dma_start(w2h[:, a:b], W2H[:, a:b])
                nc.sync.dma_start(w2l[:, a:b], W2L[:, a:b])

            def emit_mm2(h_hi, h_lo, t0, tt, k2, k3, mid=None, post=None):
                # Graded fp8 mm2: full (w2h,h_hi) pass + (w2l,h_hi) over
                # [:k2] + (w2h,h_lo) over [:k3]; b2 added at the evict.
                # mid() runs after chain 5 (the next tile's stats-broadcast
                # matmuls); post(j) runs twice per chain from chain 6 (the
                # next tile's normalize/GELU work). With no post work (final
                # tile) the evict alternates DVE/ACT so neither throttles the
                # short chains.
                step = 0
                passes = [(w2h, h_hi, tt)]
                if k2:
                    passes.append((w2l, h_hi, k2))
                if k3:
                    passes.append((w2h, h_lo, k3))
                npass = len(passes)
                ot = None
                for k in range(NK):
                    pm = ps_mm.tile([P, TT], f32, tag="mm", name="mm2")[:, :tt]
                    for pi, (wt, ht, g) in enumerate(passes):
                        for kp in range(KP):
                            nc.tensor.matmul(
                                pm[:, :g],
                                wt[:, k, kp, :, :],
                                ht[:, kp, :, :g],
                                start=(pi == 0 and kp == 0),
                                stop=(pi == npass - 1 and kp == KP - 1),
                                perf_mode=DR,
                            )
                    if k % 2 == 0:
                        # batch 2 output chunks per DMA: halves the number of
                        # HWDGE acquisitions (the exclusive HWDGE device
                        # serializes the kernel drain otherwise)
                        ot = op.tile([P, 2, tt], bf16, tag=f"out{tt}",
                                     name="out", bufs=(2 if tt == TT else 5))
                    nc.vector.tensor_scalar(
                        ot[:, k % 2, :], pm[:], 1.0 / SW2, b2s[:, k : k + 1],
                        Mul, Add,
                    )
                    if k % 2 == 1:
                        nc.sync.dma_start(
                            outT[:, k - 1 : k + 1, t0 : t0 + tt], ot[:]
                        )
                    if k == 5 and mid is not None:
                        mid()
                    if k >= 6 and post is not None:
                        for _ in range(2):
                            if step < NJ:
                                post(step)
                                step += 1
                while post is not None and step < NJ:
                    post(step)
                    step += 1

            prev = None
            backlog = []
            tiles = _t_tiles(C)

            def emit_x(i):
                # Prefetch tile i's activations (one tile ahead of use) so
                # the DMA never queues behind an out-DMA whose SEQ wait only
                # clears at the end of an mm2 phase.
                t0, tt = tiles[i]
                g2 = plan[i][1]
                xh = xp.tile([P, DP, 2, TT], f8, tag="xh", name="xh", bufs=2)
                nc.sync.dma_start(xh[:, :, :, :tt], XH[:, :, :, t0 : t0 + tt])
                if g2:
                    xl = xp.tile([P, DP, 2, TT], f8, tag="xl", name="xl", bufs=2)
                    nc.sync.dma_start(xl[:, :, :, :g2], XL[:, :, :, t0 : t0 + g2])
                else:
                    xl = None
                return xh, xl

            x_pref = None
            for tile_i, (t0, tt) in enumerate(tiles):
                tt_, g2, g3, k2, k3 = plan[tile_i]
                assert tt_ == tt
                if tile_i == 0:
                    emit_w1(0, 1)  # W1 j=0 ahead of x so the first chain starts fast
                    x_pref = emit_x(0)
                xh, xl = x_pref
                if tile_i == 0:
                    emit_w1(1, 3)
                    emit_const_dmas()
                h = hp.tile(
                    [P, NJ, tt], bf16, tag=f"h{tt}", name="h",
                    bufs=(2 if tt == TT else 1),
                )
                h_hi = hxp.tile(
                    [P, KP, 2, tt], f8, tag=f"hh{tt}", name="h_hi",
                    bufs=(2 if tt == TT else 1),
                )
                h_lo = hxp.tile(
                    [P, KP, 2, tt], f8, tag=f"hl{tt}", name="h_lo",
                    bufs=(2 if tt == TT else 1),
                ) if k3 else None
                s_ps = ps_acc.tile([32, TT], f32, tag="sacc", name="sacc")[:, :tt]
                q_ps = ps_acc.tile([32, TT], f32, tag="qacc", name="qacc")[:, :tt]

                # ---- graded mm1; Q ones-matmuls deferred one pair so the PE
                # never waits on the ACT evict / DVE square chain; tile0's
                # norm/GELU backlog drip-fed through tile1's loop ----
                m1_passes = [(w1h, xh, tt)]
                if g2:
                    m1_passes.append((w1h, xl, g2))
                if g3:
                    m1_passes.append((w1l, xh, g3))
                np1 = len(m1_passes)
                pend_q = None
                sq = None
                for j in range(NJ):
                    if tile_i == 0:
                        if j == 0:
                            emit_w1(3, 8)
                        elif j == 4:
                            emit_w1(8, NJ)
                        elif j == 8:
                            emit_w2(0, 4)
                        elif j == 12:
                            emit_w2(4, 8)
                    elif tile_i == 1:
                        if j == 0:
                            emit_w2(8, 12)
                        elif j == 4:
                            emit_w2(12, NK)
                        elif j == 8:
                            nc.sync.dma_start(b2s[:], b2[:])
                    pm = ps_mm.tile([P, TT], f32, tag="mm", name="mm1")[:, :tt]
                    for pi, (wt, xt, g) in enumerate(m1_passes):
                        for dp in range(DP):
                            nc.tensor.matmul(
                                pm[:, :g],
                                wt[:, j, dp, :, :],
                                xt[:, dp, :, :g],
                                start=(pi == 0 and dp == 0),
                                stop=(pi == np1 - 1 and dp == DP - 1),
                                perf_mode=DR,
                            )
                    nc.scalar.activation(
                        h[:, j, :], pm[:], Ident,
                        bias=b1s[:, j : j + 1], scale=1.0 / (SX * SW1),
                    )
                    if j % 2 == 0:
                        sq = sqp.tile([P, 2, TT], f8, tag="sq", name="sq")
                    nc.vector.tensor_mul(sq[:, j % 2, :tt], h[:, j, :], h[:, j, :])
                    if j % 2 == 1:
                        if pend_q is not None:
                            jp, sqt = pend_q
                            nc.tensor.matmul(
                                q_ps[:], ones_q[:], sqt[:, :, :tt],
                                start=(jp == 0), stop=(jp == NJ // 2 - 1),
                                perf_mode=DR,
                            )
                        pend_q = (j // 2, sq)
                    if backlog:
                        backlog.pop(0)()
                while backlog:  # finish tile0's backlog before mm2(0) reads h
                    backlog.pop(0)()
                # S-fold: the W1-column-sum row, 1-pass DR into s_ps
                for dp in range(DP):
                    nc.tensor.matmul(
                        s_ps[:],
                        w1sh[:, dp, :, :],
                        xh[:, dp, :, :tt],
                        start=(dp == 0),
                        stop=(dp == DP - 1),
                        perf_mode=DR,
                    )
                jp, sqt = pend_q
                nc.tensor.matmul(
                    q_ps[:], ones_q[:], sqt[:, :, :tt],
                    start=(jp == 0), stop=(jp == NJ // 2 - 1),
                    perf_mode=DR,
                )

                # ---- LN stats (DVE/ACT only; broadcasts happen mid-mm2) ----
                mu = statp.tile([1, TT], f32, tag="mu", name="mu")[:, :tt]
                nc.vector.tensor_scalar(
                    mu[:], s_ps[0:1, :], 1.0 / (SX * SW1S * H), sb1h[:], Mul, Add
                )
                tmp = statp.tile([1, TT], f32, tag="tmp", name="tmp")[:, :tt]
                nc.vector.tensor_scalar_mul(tmp[:], q_ps[0:1, :], 1.0 / H)
                tmp2 = statp.tile([1, TT], f32, tag="tmp2", name="tmp2")[:, :tt]
                nc.vector.tensor_mul(tmp2[:], mu[:], mu[:])
                nc.vector.tensor_sub(tmp[:], tmp[:], tmp2[:])          # var
                nc.vector.tensor_scalar(tmp2[:], tmp[:], ya[:], yb[:], Mul, Add)
                nc.vector.tensor_mul(tmp[:], tmp[:], tmp2[:])
                nc.vector.tensor_mul(tmp[:], tmp[:], tmp2[:])
                nc.vector.tensor_scalar(tmp[:], tmp[:], -0.5, 1.5, Mul, Add)
                nc.vector.tensor_mul(tmp[:], tmp[:], tmp2[:])          # rstd
                a_row = statp.tile([1, TT], bf16, tag="a_row", name="a_row", bufs=2)
                nc.vector.tensor_copy(a_row[:, :tt], tmp[:])
                b_row = statp.tile([1, TT], bf16, tag="b_row", name="b_row", bufs=2)
                nc.vector.tensor_mul(b_row[:, :tt], mu[:], tmp[:])

                a_sb = statp.tile([P, TT], bf16, tag="a_sb", name="a_sb", bufs=2)
                b_sb = statp.tile([P, TT], bf16, tag="b_sb", name="b_sb", bufs=2)

                def emit_bc(a_row=a_row, b_row=b_row, a_sb=a_sb, b_sb=b_sb, tt=tt):
                    # per-token stat rows -> all partitions, on the idle
                    # GPSIMD engine (frees the PE matmuls, the DVE
                    # PSUM-copies, and two PSUM banks)
                    nc.gpsimd.partition_broadcast(a_sb[:, :tt], a_row[:, :tt])
                    nc.gpsimd.partition_broadcast(b_sb[:, :tt], b_row[:, :tt])

                def emit_norm_gelu(
                    j, h=h, h_hi=h_hi, h_lo=h_lo, a_sb=a_sb, b_sb=b_sb, tt=tt,
                    k3=k3, spread=False,
                ):
                    # normalize (DVE bf16) + GELU (ACT: fp8 h_hi full width;
                    # bf16 + h_lo residual only over [:k3]). In spread mode
                    # (tile0 backlog, no mm2 window to hide in) the h_lo path
                    # uses a Pool copy and alternating Pool/DVE subs so the
                    # work balances across all three engines.
                    jp, pl = j // 2, j % 2
                    hj = h[:, j, :tt]
                    nc.vector.tensor_mul(hj, hj, a_sb[:, :tt])
                    nc.vector.tensor_sub(hj, hj, b_sb[:, :tt])
                    if spread and (j % 8 == 7 or not k3):
                        # rebalance: these backlog items skip the Pool copy
                        # (Pool is the overloaded engine in the first tile's
                        # backlog window); gelu straight into h_hi on ACT
                        spread = False
                    if spread:
                        nc.scalar.activation(
                            hj, hj, Gelu,
                            bias=lbs[:, j : j + 1], scale=lgs[:, j : j + 1],
                        )
                        nc.gpsimd.tensor_copy(h_hi[:, jp, pl, :tt], hj)
                        if k3:
                            eng = nc.gpsimd if j % 2 else nc.vector
                            eng.tensor_sub(
                                h_lo[:, jp, pl, :k3], hj[:, :k3], h_hi[:, jp, pl, :k3]
                            )
                        return
                    nc.scalar.activation(
                        h_hi[:, jp, pl, :tt], hj, Gelu,
                        bias=lbs[:, j : j + 1], scale=lgs[:, j : j + 1],
                    )
                    if k3:
                        nc.scalar.activation(
                            hj[:, :k3], hj[:, :k3], Gelu,
                            bias=lbs[:, j : j + 1], scale=lgs[:, j : j + 1],
                        )
                        nc.vector.tensor_sub(
                            h_lo[:, jp, pl, :k3], hj[:, :k3], h_hi[:, jp, pl, :k3]
                        )

                if tile_i + 1 < len(tiles):
                    x_pref = emit_x(tile_i + 1)

                # ---- previous tile's mm2 on the PE, with this tile's
                # broadcasts at chain 5 and norm/GELU from chain 6 ----
                if prev is not None:
                    emit_mm2(*prev, mid=emit_bc, post=emit_norm_gelu)
                else:
                    backlog.append(emit_bc)
                    backlog.extend(
                        (lambda j=j, f=emit_norm_gelu: f(j, spread=True))
                        for j in range(NJ)
                    )
                prev = (h_hi, h_lo, t0, tt, k2, k3)

            if len(tiles) == 1:  # safety for tiny C: no tile-1 DMA slots
                emit_w2(8, NK)
                nc.sync.dma_start(b2s[:], b2[:])
            for fn in backlog:
                fn()
            emit_mm2(*prev)

    nc.compile()
    return nc


def _route(x64, Wg64, bg64):
    """Host gating: per-token top-2 expert ids and renormalized weights."""
    logits = x64 @ Wg64 + bg64                      # [N, E] fp64
    order = np.argsort(-logits, axis=1, kind="stable")[:, :TOPK]
    l0 = np.take_along_axis(logits, order, axis=1)  # [N, 2] descending
    w0 = 1.0 / (1.0 + np.exp(l0[:, 1] - l0[:, 0]))
    w = np.stack([w0, 1.0 - w0], axis=1)
    return order, w


def _split8(a):
    hi = a.astype(F8)
    lo = (a - hi.astype(np.float32)).astype(F8)
    return hi, lo


def kernel(x, W1, b1, ln_g, ln_b, W2, b2, Wg, bg):
    x = np.ascontiguousarray(np.asarray(x, dtype=np.float32))
    W1 = np.asarray(W1, dtype=np.float32)
    b1 = np.asarray(b1, dtype=np.float32)
    ln_g = np.asarray(ln_g, dtype=np.float32)
    ln_b = np.asarray(ln_b, dtype=np.float32)
    W2 = np.asarray(W2, dtype=np.float32)
    b2 = np.asarray(b2, dtype=np.float32)
    Wg = np.asarray(Wg, dtype=np.float32)
    bg = np.asarray(bg, dtype=np.float32)
    N = x.shape[0]

    order, w = _route(x.astype(np.float64), Wg.astype(np.float64), bg.astype(np.float64))

    tok_idx, tok_w = [], []
    for e in range(E):
        sel = np.nonzero((order[:, 0] == e) | (order[:, 1] == e))[0]
        we = np.where(order[sel, 0] == e, w[sel, 0], w[sel, 1]).astype(np.float32)
        o = np.argsort(-we, kind="stable")   # high-combine-weight slots first
        tok_idx.append(sel[o])
        tok_w.append(we[o])
    C = max(GRAN, int(-(-max(len(s) for s in tok_idx) // GRAN)) * GRAN)

    # normalized w^2 slot profile -> graded pass plan (shared by all cores)
    u = np.zeros(C)
    for e in range(E):
        u[: len(tok_w[e])] += tok_w[e].astype(np.float64) ** 2
    u /= u.sum()
    plan = _plan(u)

    key = (C, plan)
    if key not in _kernel_cache:
        _kernel_cache[key] = _build(C, plan)
    nc = _kernel_cache[key]

    in_maps = []
    for e in range(E):
        idx = np.zeros(C, dtype=np.int64)
        idx[: len(tok_idx[e])] = tok_idx[e]
        xg = x[idx] * SX                              # [C, D]
        xg[len(tok_idx[e]):] = 0.0
        xh, xl = _split8(xg)
        # [C, D] -> [P, DP, 2, C]
        xh_d = np.ascontiguousarray(xh.reshape(C, DP, 2, P).transpose(3, 1, 2, 0))
        xl_d = np.ascontiguousarray(xl.reshape(C, DP, 2, P).transpose(3, 1, 2, 0))
        w1h, w1l = _split8(W1[e] * SW1)               # [D, H]
        w1h_d = np.ascontiguousarray(
            w1h.reshape(DP, 2, P, NJ, P).transpose(2, 3, 0, 1, 4)
        )
        w1l_d = np.ascontiguousarray(
            w1l.reshape(DP, 2, P, NJ, P).transpose(2, 3, 0, 1, 4)
        )
        # S-fold: column-sum of W1 (scaled), replicated over 32 lhsT columns
        w1s = W1[e].sum(axis=1) * SW1S                # [D]
        w1sh, _ = _split8(w1s)
        w1sh_d = np.ascontiguousarray(np.broadcast_to(
            w1sh.reshape(DP, 2, P).transpose(2, 0, 1)[:, :, :, None], (P, DP, 2, 32)
        ).astype(F8))
        sb1h_d = np.full((1, 1), b1[e].sum() / H, dtype=np.float32)
        w1c = W1[e] - W1[e].mean(axis=1, keepdims=True)
        vbar = float((w1c * w1c).sum() / H + np.var(b1[e]))
        y0 = 1.0 / np.sqrt(vbar + LN_EPS)
        ya_d = np.full((1, 1), -0.5 * y0 ** 3, dtype=np.float32)
        yb_d = np.full((1, 1), 1.5 * y0, dtype=np.float32)
        w2h, w2l = _split8(W2[e] * SW2)               # [H, H]
        w2h_d = np.ascontiguousarray(
            w2h.reshape(KP, 2, P, NK, P).transpose(2, 3, 0, 1, 4)
        )
        w2l_d = np.ascontiguousarray(
            w2l.reshape(KP, 2, P, NK, P).transpose(2, 3, 0, 1, 4)
        )
        in_maps.append(
            {
                "XH": xh_d,
                "XL": xl_d,
                "W1H": w1h_d,
                "W1L": w1l_d,
                "W1SH": w1sh_d,
                "SB1H": sb1h_d,
                "YA": ya_d,
                "YB": yb_d,
                "W2H": w2h_d,
                "W2L": w2l_d,
                "b1": np.ascontiguousarray(b1[e].reshape(NJ, P).T),
                "b2": np.ascontiguousarray(b2[e].reshape(NK, P).T),
                "lg": np.ascontiguousarray(ln_g[e].reshape(NJ, P).T),
                "lb": np.ascontiguousarray(ln_b[e].reshape(NJ, P).T),
            }
        )

    results = _run(key, nc, in_maps)

    y = np.zeros((N, H), dtype=np.float32)
    for e in range(E):
        cnt = len(tok_idx[e])
        eoT = (
            results[e]["outT"].transpose(1, 0, 2).reshape(H, C).astype(np.float32)
        )
        y[tok_idx[e]] += tok_w[e][:, None] * eoT[:, :cnt].T
    return y


_neff_cache: dict[tuple, str] = {}


def _run(key, nc, in_maps):
    C = key[0]
    if axon_active():
        # PJRT path; NEFF compile is cached by libneuronxla.
        return run_bass_kernel_spmd(nc, in_maps, core_ids=list(range(E))).results
    # Native path: compile once per capacity, then execute the cached NEFF.
    from concourse.bass_utils import compile_bass_kernel, run_neff

    if key not in _neff_cache:
        _neff_cache[key] = compile_bass_kernel(nc, tempfile.mkdtemp())
    out_maps = [{"outT": np.zeros((P, NK, C), dtype=BF)} for _ in range(E)]
    in_maps = [m.copy() for m in in_maps]
    if nc.partition_id_tensor:
        for core_id, m in enumerate(in_maps):
            m[nc.partition_id_tensor.name] = np.array([[core_id]], dtype=np.uint32)
    return run_neff(
        _neff_cache[key],
        in_maps,
        out_maps,
        core_ids=list(range(E)),
        has_collectives=False,
    )


# revision 5
# speedup vs baseline: 1.1738x; 1.0035x over previous
# MoE (top-2 of 8 experts) kernel for 8 Trainium2 NeuronCores.
#
# Strategy: expert-parallel sparse routing with fp8 DoubleRow matmuls and
# GRADED PRECISION. Host computes the gating network and per-expert token
# lists; core e runs expert e's FFN (x@W1+b1 -> LayerNorm -> erf-GELU ->
# @W2+b2) on its routed tokens, sorted by combine weight (descending).
# Matmuls run as fp8(e4m3) DoubleRow with hi/lo error compensation, but the
# compensation passes cover only a column prefix of each (w-sorted) token
# tile: tokens with large combine weights get full 3-pass accuracy, tokens
# with small weights (and padding) get 1-2 passes. The per-slot pass plan is
# chosen by a Lagrangian knapsack on the w^2 profile so the end-to-end
# rel err stays under a fixed budget while minimizing PE cycles.
# All weights live in SBUF (10MB fp8), loaded once in contiguous per-chunk
# DMAs. LayerNorm S-sums come from a 1-pass W1-column-sum lhsT row; Q-sums
# use a paired-fp8 DoubleRow ones-matmul on h^2; per-token stats broadcast
# across partitions with K=1 f32r matmuls, emitted mid-mm2 so the PE never
# waits on the stats chain. The first tile's normalize/GELU backlog is
# drip-fed through the second tile's mm1 loop.

import tempfile

import ml_dtypes
import numpy as np

import concourse.bacc as bacc
import concourse.mybir as mybir
import concourse.tile as tile
from concourse._compat import axon_active
from concourse.bass_utils import run_bass_kernel_spmd

P = 128
D, H, E, TOPK = 1024, 2048, 8, 2
DP, KP, NJ, NK = D // 256, H // 256, H // P, H // P  # 4, 8, 16, 16
LN_EPS = 1e-5
TT = 512           # main token tile
GRAN = 16          # capacity granularity
SX, SW1, SW2 = 16.0, 256.0, 256.0   # fp8 pre-quantization scales
SW1S = 32.0        # scale for the W1 column-sum row (S-fold)
F8 = ml_dtypes.float8_e4m3
BF = ml_dtypes.bfloat16

# Error model for the pass planner (err^2 contributions, measured on the
# reference input distribution; used as a heuristic for any input).
ERR_TARGET = 1.66e-2
E1_2, E1_1 = 5.98e-4, 13.49e-4   # mm1 at 2 passes / 1 pass
E2_2, E2_1 = 7.03e-4, 14.40e-4   # mm2 at 2 passes / 1 pass
FLOOR2 = 0.23e-4
C1_NS = 16 * 4 * 0.5 / 2.4       # PE ns/slot for one extra mm1 pass
C2_NS = 16 * 8 * 0.5 / 2.4

_kernel_cache: dict[tuple, object] = {}


def _t_tiles(C):
    tiles, t0 = [], 0
    while t0 < C:
        tt = TT if C - t0 >= TT else C - t0
        tiles.append((t0, tt))
        t0 += tt
    # Tail tile last: its cheap mm2 is the only un-overlapped one, and
    # full-size norm/GELU phases pair with full-size mm2 phases.
    return tiles


def _plan(u):
    """Per-slot pass levels (p, q) for mm1/mm2 given the normalized w^2 slot
    profile u, then snapped to per-tile compensation widths."""
    C = len(u)

    def plan_at(lam):
        p = np.full(C, 3, dtype=np.int64)
        q = np.full(C, 3, dtype=np.int64)
        p[u <= C1_NS / (lam * E1_2)] = 2
        p[(p == 2) & (u <= C1_NS / (lam * (E1_1 - E1_2)))] = 1
        q[u <= C2_NS / (lam * E2_2)] = 2
        q[(q == 2) & (u <= C2_NS / (lam * (E2_1 - E2_2)))] = 1
        return p, q

    def err2(p, q):
        t = np.zeros(C)
        t[p == 2] += E1_2
        t[p == 1] += E1_1
        t[q == 2] += E2_2
        t[q == 1] += E2_1
        return float((u * t).sum())

    B = ERR_TARGET * ERR_TARGET - FLOOR2
    lo, hi = 1e0, 1e14
    for _ in range(100):
        lam = (lo * hi) ** 0.5
        p, q = plan_at(lam)
        if err2(p, q) > B:
            lo = lam   # too much error -> fewer reductions needed
        else:
            hi = lam
    p, q = plan_at(hi)

    tiles = []
    for (t0, tt) in _t_tiles(C):
        def width(lev, need):
            n = int((lev[t0 : t0 + tt] >= need).sum())
            if n == 0:
                return 0
            n = min(tt, (n + 31) // 32 * 32)
            return max(n, min(tt, 128))
        g2, g3 = width(p, 2), width(p, 3)
        k2, k3 = width(q, 2), width(q, 3)
        tiles.append((tt, g2, g3, k2, k3))
    return tuple(tiles)


def _build(C: int, plan):
    f32, f32r, bf16, f8 = (
        mybir.dt.float32, mybir.dt.float32r, mybir.dt.bfloat16, mybir.dt.float8e4
    )
    DR = mybir.MatmulPerfMode.DoubleRow
    Mul, Add = mybir.AluOpType.mult, mybir.AluOpType.add
    nc = bacc.Bacc("TRN2", target_bir_lowering=False, debug=False, num_devices=8)
    XH = nc.dram_tensor("XH", [P, DP, 2, C], f8, kind="ExternalInput").ap()
    XL = nc.dram_tensor("XL", [P, DP, 2, C], f8, kind="ExternalInput").ap()
    W1H = nc.dram_tensor("W1H", [P, NJ, DP, 2, P], f8, kind="ExternalInput").ap()
    W1L = nc.dram_tensor("W1L", [P, NJ, DP, 2, P], f8, kind="ExternalInput").ap()
    W1SH = nc.dram_tensor("W1SH", [P, DP, 2, 32], f8, kind="ExternalInput").ap()
    SB1H = nc.dram_tensor("SB1H", [1, 1], f32, kind="ExternalInput").ap()
    W2H = nc.dram_tensor("W2H", [P, NK, KP, 2, P], f8, kind="ExternalInput").ap()
    W2L = nc.dram_tensor("W2L", [P, NK, KP, 2, P], f8, kind="ExternalInput").ap()
    YA = nc.dram_tensor("YA", [1, 1], f32, kind="ExternalInput").ap()
    YB = nc.dram_tensor("YB", [1, 1], f32, kind="ExternalInput").ap()
    b1 = nc.dram_tensor("b1", [P, NJ], f32, kind="ExternalInput").ap()
    b2 = nc.dram_tensor("b2", [P, NK], f32, kind="ExternalInput").ap()
    lg = nc.dram_tensor("lg", [P, NJ], f32, kind="ExternalInput").ap()
    lb = nc.dram_tensor("lb", [P, NJ], f32, kind="ExternalInput").ap()
    outT = nc.dram_tensor("outT", [P, NK, C], bf16, kind="ExternalOutput").ap()

    Gelu = mybir.ActivationFunctionType.Gelu
    Sqrt = mybir.ActivationFunctionType.Sqrt
    Ident = mybir.ActivationFunctionType.Identity

    with tile.TileContext(nc) as tc:
        with (
            tc.tile_pool(name="const", bufs=1) as constp,
            tc.tile_pool(name="wp", bufs=1) as wp,
            tc.tile_pool(name="xp", bufs=1) as xp,
            tc.tile_pool(name="hp", bufs=2) as hp,
            tc.tile_pool(name="hxp", bufs=2) as hxp,
            tc.tile_pool(name="sqp", bufs=2) as sqp,
            tc.tile_pool(name="op", bufs=2) as op,
            tc.tile_pool(name="statp", bufs=1) as statp,
            tc.tile_pool(name="ps_mm", bufs=6, space="PSUM") as ps_mm,
            tc.tile_pool(name="ps_acc", bufs=1, space="PSUM") as ps_acc,
        ):
            b1s = constp.tile([P, NJ], f32)
            b2s = constp.tile([P, NK], f32)
            lgs = constp.tile([P, NJ], f32)
            lbs = constp.tile([P, NJ], f32)
            w1sh = constp.tile([P, DP, 2, 32], f8)
            sb1h = constp.tile([1, 1], f32)
            ya = constp.tile([1, 1], f32)
            yb = constp.tile([1, 1], f32)

            def emit_const_dmas():
                nc.sync.dma_start(b1s[:], b1[:])
                nc.sync.dma_start(lgs[:], lg[:])
                nc.sync.dma_start(lbs[:], lb[:])
                nc.sync.dma_start(w1sh[:], W1SH[:])
                nc.sync.dma_start(sb1h[:], SB1H[:])
                nc.sync.dma_start(ya[:], YA[:])
                nc.sync.dma_start(yb[:], YB[:])

            ones_q = constp.tile([P, 2, 32], f8)   # lhsT for Q paired DR sums
            nc.any.memset(ones_q[:], 1.0)
            eps_t = constp.tile([1, 1], f32)
            nc.any.memset(eps_t[:], LN_EPS)

            # fp8 weights, SBUF-resident for the whole kernel, streamed in
            # contiguous per-chunk DMAs staged around the first two tiles.
            w1h = wp.tile([P, NJ, DP, 2, P], f8)
            w1l = wp.tile([P, NJ, DP, 2, P], f8)
            w2h = wp.tile([P, NK, KP, 2, P], f8)
            w2l = wp.tile([P, NK, KP, 2, P], f8)

            def emit_w1(a, b):
                nc.sync.dma_start(w1h[:, a:b], W1H[:, a:b])
                nc.sync.dma_start(w1l[:, a:b], W1L[:, a:b])

            def emit_w2(a, b):
                nc.sync.dma_start(w2h[:, a:b], W2H[:, a:b])
                nc.sync.dma_start(w2l[:, a:b], W2L[:, a:b])

            def emit_mm2(h_hi, h_lo, t0, tt, k2, k3, mid=None, post=None):
                # Graded fp8 mm2: full (w2h,h_hi) pass + (w2l,h_hi) over
                # [:k2] + (w2h,h_lo) over [:k3]; b2 added at the evict.
                # mid() runs after chain 5 (the next tile's stats-broadcast
                # matmuls); post(j) runs twice per chain from chain 6 (the
                # next tile's normalize/GELU work). With no post work (final
                # tile) the evict alternates DVE/ACT so neither throttles the
                # short chains.
                step = 0
                passes = [(w2h, h_hi, tt)]
                if k2:
                    passes.append((w2l, h_hi, k2))
                if k3:
                    passes.append((w2h, h_lo, k3))
                npass = len(passes)
                ot = None
                for k in range(NK):
                    pm = ps_mm.tile([P, TT], f32, tag="mm", name="mm2")[:, :tt]
                    for pi, (wt, ht, g) in enumerate(passes):
                        for kp in range(KP):
                            nc.tensor.matmul(
                                pm[:, :g],
                                wt[:, k, kp, :, :],
                                ht[:, kp, :, :g],
                                start=(pi == 0 and kp == 0),
                                stop=(pi == npass - 1 and kp == KP - 1),
                                perf_mode=DR,
                            )
                    if k % 2 == 0:
                        # batch 2 output chunks per DMA: halves the number of
                        # HWDGE acquisitions (the exclusive HWDGE device
                        # serializes the kernel drain otherwise)
                        ot = op.tile([P, 2, tt], bf16, tag=f"out{tt}",
                                     name="out", bufs=(2 if tt == TT else 5))
                    nc.vector.tensor_scalar(
                        ot[:, k % 2, :], pm[:], 1.0 / SW2, b2s[:, k : k + 1],
                        Mul, Add,
                    )
                    if k % 2 == 1:
                        nc.sync.dma_start(
                            outT[:, k - 1 : k + 1, t0 : t0 + tt], ot[:]
                        )
                    if k == 5 and mid is not None:
                        mid()
                    if k >= 6 and post is not None:
                        for _ in range(2):
                            if step < NJ:
                                post(step)
                                step += 1
                while post is not None and step < NJ:
                    post(step)
                    step += 1

            prev = None
            backlog = []
            tiles = _t_tiles(C)

            def emit_x(i):
                # Prefetch tile i's activations (one tile ahead of use) so
                # the DMA never queues behind an out-DMA whose SEQ wait only
                # clears at the end of an mm2 phase.
                t0, tt = tiles[i]
                g2 = plan[i][1]
                xh = xp.tile([P, DP, 2, TT], f8, tag="xh", name="xh", bufs=2)
                nc.sync.dma_start(xh[:, :, :, :tt], XH[:, :, :, t0 : t0 + tt])
                if g2:
                    xl = xp.tile([P, DP, 2, TT], f8, tag="xl", name="xl", bufs=2)
                    nc.sync.dma_start(xl[:, :, :, :g2], XL[:, :, :, t0 : t0 + g2])
                else:
                    xl = None
                return xh, xl

            x_pref = None
            for tile_i, (t0, tt) in enumerate(tiles):
                tt_, g2, g3, k2, k3 = plan[tile_i]
                assert tt_ == tt
                if tile_i == 0:
                    emit_w1(0, 1)  # W1 j=0 ahead of x so the first chain starts fast
                    x_pref = emit_x(0)
                xh, xl = x_pref
                if tile_i == 0:
                    emit_w1(1, 3)
                    emit_const_dmas()
                h = hp.tile(
                    [P, NJ, tt], bf16, tag=f"h{tt}", name="h",
                    bufs=(2 if tt == TT else 1),
                )
                h_hi = hxp.tile(
                    [P, KP, 2, tt], f8, tag=f"hh{tt}", name="h_hi",
                    bufs=(2 if tt == TT else 1),
                )
                h_lo = hxp.tile(
                    [P, KP, 2, tt], f8, tag=f"hl{tt}", name="h_lo",
                    bufs=(2 if tt == TT else 1),
                ) if k3 else None
                s_ps = ps_acc.tile([32, TT], f32, tag="sacc", name="sacc")[:, :tt]
                q_ps = ps_acc.tile([32, TT], f32, tag="qacc", name="qacc")[:, :tt]

                # ---- graded mm1; Q ones-matmuls deferred one pair so the PE
                # never waits on the ACT evict / DVE square chain; tile0's
                # norm/GELU backlog drip-fed through tile1's loop ----
                m1_passes = [(w1h, xh, tt)]
                if g2:
                    m1_passes.append((w1h, xl, g2))
                if g3:
                    m1_passes.append((w1l, xh, g3))
                np1 = len(m1_passes)
                pend_q = None
                sq = None
                for j in range(NJ):
                    if tile_i == 0:
                        if j == 0:
                            emit_w1(3, 8)
                        elif j == 4:
                            emit_w1(8, NJ)
                        elif j == 8:
                            emit_w2(0, 4)
                        elif j == 12:
                            emit_w2(4, 8)
                    elif tile_i == 1:
                        if j == 0:
                            emit_w2(8, 12)
                        elif j == 4:
                            emit_w2(12, NK)
                        elif j == 8:
                            nc.sync.dma_start(b2s[:], b2[:])
                    pm = ps_mm.tile([P, TT], f32, tag="mm", name="mm1")[:, :tt]
                    for pi, (wt, xt, g) in enumerate(m1_passes):
                        for dp in range(DP):
                            nc.tensor.matmul(
                                pm[:, :g],
                                wt[:, j, dp, :, :],
                                xt[:, dp, :, :g],
                                start=(pi == 0 and dp == 0),
                                stop=(pi == np1 - 1 and dp == DP - 1),
                                perf_mode=DR,
                            )
                    nc.scalar.activation(
                        h[:, j, :], pm[:], Ident,
                        bias=b1s[:, j : j + 1], scale=1.0 / (SX * SW1),
                    )
                    if j % 2 == 0:
                        sq = sqp.tile([P, 2, TT], f8, tag="sq", name="sq")
                    nc.vector.tensor_mul(sq[:, j % 2, :tt], h[:, j, :], h[:, j, :])
                    if j % 2 == 1:
                        if pend_q is not None:
                            jp, sqt = pend_q
                            nc.tensor.matmul(
                                q_ps[:], ones_q[:], sqt[:, :, :tt],
                                start=(jp == 0), stop=(jp == NJ // 2 - 1),
                                perf_mode=DR,
                            )
                        pend_q = (j // 2, sq)
                    if backlog:
                        backlog.pop(0)()
                while backlog:  # finish tile0's backlog before mm2(0) reads h
                    backlog.pop(0)()
                # S-fold: the W1-column-sum row, 1-pass DR into s_ps
                for dp in range(DP):
                    nc.tensor.matmul(
                        s_ps[:],
                        w1sh[:, dp, :, :],
                        xh[:, dp, :, :tt],
                        start=(dp == 0),
                        stop=(dp == DP - 1),
                        perf_mode=DR,
                    )
                jp, sqt = pend_q
                nc.tensor.matmul(
                    q_ps[:], ones_q[:], sqt[:, :, :tt],
                    start=(jp == 0), stop=(jp == NJ // 2 - 1),
                    perf_mode=DR,
                )

                # ---- LN stats (DVE/ACT only; broadcasts happen mid-mm2) ----
                mu = statp.tile([1, TT], f32, tag="mu", name="mu")[:, :tt]
                nc.vector.tensor_scalar(
                    mu[:], s_ps[0:1, :], 1.0 / (SX * SW1S * H), sb1h[:], Mul, Add
                )
                tmp = statp.tile([1, TT], f32, tag="tmp", name="tmp")[:, :tt]
                nc.vector.tensor_scalar_mul(tmp[:], q_ps[0:1, :], 1.0 / H)
                tmp2 = statp.tile([1, TT], f32, tag="tmp2", name="tmp2")[:, :tt]
                nc.vector.tensor_mul(tmp2[:], mu[:], mu[:])
                nc.vector.tensor_sub(tmp[:], tmp[:], tmp2[:])          # var
                nc.vector.tensor_scalar(tmp2[:], tmp[:], ya[:], yb[:], Mul, Add)
                nc.vector.tensor_mul(tmp[:], tmp[:], tmp2[:])
                nc.vector.tensor_mul(tmp[:], tmp[:], tmp2[:])
                nc.vector.tensor_scalar(tmp[:], tmp[:], -0.5, 1.5, Mul, Add)
                nc.vector.tensor_mul(tmp[:], tmp[:], tmp2[:])          # rstd
                a_row = statp.tile([1, TT], bf16, tag="a_row", name="a_row", bufs=2)
                nc.vector.tensor_copy(a_row[:, :tt], tmp[:])
                b_row = statp.tile([1, TT], bf16, tag="b_row", name="b_row", bufs=2)
                nc.vector.tensor_mul(b_row[:, :tt], mu[:], tmp[:])

                a_sb = statp.tile([P, TT], bf16, tag="a_sb", name="a_sb", bufs=2)
                b_sb = statp.tile([P, TT], bf16, tag="b_sb", name="b_sb", bufs=2)

                def emit_bc(a_row=a_row, b_row=b_row, a_sb=a_sb, b_sb=b_sb, tt=tt):
                    # per-token stat rows -> all partitions, on the idle
                    # GPSIMD engine (frees the PE matmuls, the DVE
                    # PSUM-copies, and two PSUM banks)
                    nc.gpsimd.partition_broadcast(a_sb[:, :tt], a_row[:, :tt])
                    nc.gpsimd.partition_broadcast(b_sb[:, :tt], b_row[:, :tt])

                def emit_norm_gelu(
                    j, h=h, h_hi=h_hi, h_lo=h_lo, a_sb=a_sb, b_sb=b_sb, tt=tt,
                    k3=k3, spread=False,
                ):
                    # normalize (DVE bf16) + GELU (ACT: fp8 h_hi full width;
                    # bf16 + h_lo residual only over [:k3]). In spread mode
                    # (tile0 backlog, no mm2 window to hide in) the h_lo path
                    # uses a Pool copy and alternating Pool/DVE subs so the
                    # work balances across all three engines.
                    jp, pl = j // 2, j % 2
                    hj = h[:, j, :tt]
                    nc.vector.tensor_mul(hj, hj, a_sb[:, :tt])
                    nc.vector.tensor_sub(hj, hj, b_sb[:, :tt])
                    if spread and (j % 8 == 7 or not k3):
                        # rebalance: these backlog items skip the Pool copy
                        # (Pool is the overloaded engine in the first tile's
                        # backlog window); gelu straight into h_hi on ACT
                        spread = False
                    if spread:
                        nc.scalar.activation(
                            hj, hj, Gelu,
                            bias=lbs[:, j : j + 1], scale=lgs[:, j : j + 1],
                        )
                        nc.gpsimd.tensor_copy(h_hi[:, jp, pl, :tt], hj)
                        if k3:
                            eng = nc.gpsimd if j % 2 else nc.vector
                            eng.tensor_sub(
                                h_lo[:, jp, pl, :k3], hj[:, :k3], h_hi[:, jp, pl, :k3]
                            )
                        return
                    nc.scalar.activation(
                        h_hi[:, jp, pl, :tt], hj, Gelu,
                        bias=lbs[:, j : j + 1], scale=lgs[:, j : j + 1],
                    )
                    if k3:
                        nc.scalar.activation(
                            hj[:, :k3], hj[:, :k3], Gelu,
                            bias=lbs[:, j : j + 1], scale=lgs[:, j : j + 1],
                        )
                        nc.vector.tensor_sub(
                            h_lo[:, jp, pl, :k3], hj[:, :k3], h_hi[:, jp, pl, :k3]
                        )

                if tile_i + 1 < len(tiles):
                    x_pref = emit_x(tile_i + 1)

                # ---- previous tile's mm2 on the PE, with this tile's
                # broadcasts at chain 5 and norm/GELU from chain 6 ----
                if prev is not None:
                    emit_mm2(*prev, mid=emit_bc, post=emit_norm_gelu)
                else:
                    backlog.append(emit_bc)
                    backlog.extend(
                        (lambda j=j, f=emit_norm_gelu: f(j, spread=True))
                        for j in range(NJ)
                    )
                prev = (h_hi, h_lo, t0, tt, k2, k3)

            if len(tiles) == 1:  # safety for tiny C: no tile-1 DMA slots
                emit_w2(8, NK)
                nc.sync.dma_start(b2s[:], b2[:])
            for fn in backlog:
                fn()
            emit_mm2(*prev)

    nc.compile()
    return nc


def _route(x64, Wg64, bg64):
    """Host gating: per-token top-2 expert ids and renormalized weights."""
    logits = x64 @ Wg64 + bg64                      # [N, E] fp64
    order = np.argsort(-logits, axis=1, kind="stable")[:, :TOPK]
    l0 = np.take_along_axis(logits, order, axis=1)  # [N, 2] descending
    w0 = 1.0 / (1.0 + np.exp(l0[:, 1] - l0[:, 0]))
    w = np.stack([w0, 1.0 - w0], axis=1)
    return order, w


def _split8(a):
    hi = a.astype(F8)
    lo = (a - hi.astype(np.float32)).astype(F8)
    return hi, lo


def kernel(x, W1, b1, ln_g, ln_b, W2, b2, Wg, bg):
    x = np.ascontiguousarray(np.asarray(x, dtype=np.float32))
    W1 = np.asarray(W1, dtype=np.float32)
    b1 = np.asarray(b1, dtype=np.float32)
    ln_g = np.asarray(ln_g, dtype=np.float32)
    ln_b = np.asarray(ln_b, dtype=np.float32)
    W2 = np.asarray(W2, dtype=np.float32)
    b2 = np.asarray(b2, dtype=np.float32)
    Wg = np.asarray(Wg, dtype=np.float32)
    bg = np.asarray(bg, dtype=np.float32)
    N = x.shape[0]

    order, w = _route(x.astype(np.float64), Wg.astype(np.float64), bg.astype(np.float64))

    tok_idx, tok_w = [], []
    for e in range(E):
        sel = np.nonzero((order[:, 0] == e) | (order[:, 1] == e))[0]
        we = np.where(order[sel, 0] == e, w[sel, 0], w[sel, 1]).astype(np.float32)
        o = np.argsort(-we, kind="stable")   # high-combine-weight slots first
        tok_idx.append(sel[o])
        tok_w.append(we[o])
    C = max(GRAN, int(-(-max(len(s) for s in tok_idx) // GRAN)) * GRAN)

    # normalized w^2 slot profile -> graded pass plan (shared by all cores)
    u = np.zeros(C)
    for e in range(E):
        u[: len(tok_w[e])] += tok_w[e].astype(np.float64) ** 2
    u /= u.sum()
    plan = _plan(u)

    key = (C, plan)
    if key not in _kernel_cache:
        _kernel_cache[key] = _build(C, plan)
    nc = _kernel_cache[key]

    in_maps = []
    for e in range(E):
        idx = np.zeros(C, dtype=np.int64)
        idx[: len(tok_idx[e])] = tok_idx[e]
        xg = x[idx] * SX                              # [C, D]
        xg[len(tok_idx[e]):] = 0.0
        xh, xl = _split8(xg)
        # [C, D] -> [P, DP, 2, C]
        xh_d = np.ascontiguousarray(xh.reshape(C, DP, 2, P).transpose(3, 1, 2, 0))
        xl_d = np.ascontiguousarray(xl.reshape(C, DP, 2, P).transpose(3, 1, 2, 0))
        w1h, w1l = _split8(W1[e] * SW1)               # [D, H]
        w1h_d = np.ascontiguousarray(
            w1h.reshape(DP, 2, P, NJ, P).transpose(2, 3, 0, 1, 4)
        )
        w1l_d = np.ascontiguousarray(
            w1l.reshape(DP, 2, P, NJ, P).transpose(2, 3, 0, 1, 4)
        )
        # S-fold: column-sum of W1 (scaled), replicated over 32 lhsT columns
        w1s = W1[e].sum(axis=1) * SW1S                # [D]
        w1sh, _ = _split8(w1s)
        w1sh_d = np.ascontiguousarray(np.broadcast_to(
            w1sh.reshape(DP, 2, P).transpose(2, 0, 1)[:, :, :, None], (P, DP, 2, 32)
        ).astype(F8))
        sb1h_d = np.full((1, 1), b1[e].sum() / H, dtype=np.float32)
        w1c = W1[e] - W1[e].mean(axis=1, keepdims=True)
        vbar = float((w1c * w1c).sum() / H + np.var(b1[e]))
        y0 = 1.0 / np.sqrt(vbar + LN_EPS)
        ya_d = np.full((1, 1), -0.5 * y0 ** 3, dtype=np.float32)
        yb_d = np.full((1, 1), 1.5 * y0, dtype=np.float32)
        w2h, w2l = _split8(W2[e] * SW2)               # [H, H]
        w2h_d = np.ascontiguousarray(
            w2h.reshape(KP, 2, P, NK, P).transpose(2, 3, 0, 1, 4)
        )
        w2l_d = np.ascontiguousarray(
            w2l.reshape(KP, 2, P, NK, P).transpose(2, 3, 0, 1, 4)
        )
        in_maps.append(
            {
                "XH": xh_d,
                "XL": xl_d,
                "W1H": w1h_d,
                "W1L": w1l_d,
                "W1SH": w1sh_d,
                "SB1H": sb1h_d,
                "YA": ya_d,
                "YB": yb_d,
                "W2H": w2h_d,
                "W2L": w2l_d,
                "b1": np.ascontiguousarray(b1[e].reshape(NJ, P).T),
                "b2": np.ascontiguousarray(b2[e].reshape(NK, P).T),
                "lg": np.ascontiguousarray(ln_g[e].reshape(NJ, P).T),
                "lb": np.ascontiguousarray(ln_b[e].reshape(NJ, P).T),
            }
        )

    results = _run(key, nc, in_maps)

    y = np.zeros((N, H), dtype=np.float32)
    for e in range(E):
        cnt = len(tok_idx[e])
        eoT = (
            results[e]["outT"].transpose(1, 0, 2).reshape(H, C).astype(np.float32)
        )
        y[tok_idx[e]] += tok_w[e][:, None] * eoT[:, :cnt].T
    return y


_neff_cache: dict[tuple, str] = {}


def _run(key, nc, in_maps):
    C = key[0]
    if axon_active():
        # PJRT path; NEFF compile is cached by libneuronxla.
        return run_bass_kernel_spmd(nc, in_maps, core_ids=list(range(E))).results
    # Native path: compile once per capacity, then execute the cached NEFF.
    from concourse.bass_utils import compile_bass_kernel, run_neff

    if key not in _neff_cache:
        _neff_cache[key] = compile_bass_kernel(nc, tempfile.mkdtemp())
    out_maps = [{"outT": np.zeros((P, NK, C), dtype=BF)} for _ in range(E)]
    in_maps = [m.copy() for m in in_maps]
    if nc.partition_id_tensor:
        for core_id, m in enumerate(in_maps):
            m[nc.partition_id_tensor.name] = np.array([[core_id]], dtype=np.uint32)
    return run_neff(
        _neff_cache[key],
        in_maps,
        out_maps,
        core_ids=list(range(E)),
        has_collectives=False,
    )


# revision 6
# speedup vs baseline: 1.1833x; 1.0081x over previous
# MoE (top-2 of 8 experts) kernel for 8 Trainium2 NeuronCores.
#
# Strategy: expert-parallel sparse routing with fp8 DoubleRow matmuls and
# GRADED PRECISION. Host computes the gating network and per-expert token
# lists; core e runs expert e's FFN (x@W1+b1 -> LayerNorm -> erf-GELU ->
# @W2+b2) on its routed tokens, sorted by combine weight (descending).
# Matmuls run as fp8(e4m3) DoubleRow with hi/lo error compensation, but the
# compensation passes cover only a column prefix of each (w-sorted) token
# tile: tokens with large combine weights get full 3-pass accuracy, tokens
# with small weights (and padding) get 1-2 passes. The per-slot pass plan is
# chosen by a Lagrangian knapsack on the w^2 profile so the end-to-end
# rel err stays under a fixed budget while minimizing PE cycles.
# All weights live in SBUF (10MB fp8), loaded once in contiguous per-chunk
# DMAs. LayerNorm S-sums come from a 1-pass W1-column-sum lhsT row; Q-sums
# use a paired-fp8 DoubleRow ones-matmul on h^2; per-token stats broadcast
# across partitions with K=1 f32r matmuls, emitted mid-mm2 so the PE never
# waits on the stats chain. The first tile's normalize/GELU backlog is
# drip-fed through the second tile's mm1 loop.

import tempfile

import ml_dtypes
import numpy as np

import concourse.bacc as bacc
import concourse.mybir as mybir
import concourse.tile as tile
from concourse._compat import axon_active
from concourse.bass_utils import run_bass_kernel_spmd

P = 128
D, H, E, TOPK = 1024, 2048, 8, 2
DP, KP, NJ, NK = D // 256, H // 256, H // P, H // P  # 4, 8, 16, 16
LN_EPS = 1e-5
TT = 512           # main token tile
GRAN = 16          # capacity granularity
SX, SW1, SW2 = 16.0, 256.0, 256.0   # fp8 pre-quantization scales
SW1S = 32.0        # scale for the W1 column-sum row (S-fold)
F8 = ml_dtypes.float8_e4m3
BF = ml_dtypes.bfloat16

# Error model for the pass planner (err^2 contributions, measured on the
# reference input distribution; used as a heuristic for any input).
ERR_TARGET = 1.72e-2
E1_2, E1_1 = 5.98e-4, 13.49e-4   # mm1 at 2 passes / 1 pass
E2_2, E2_1 = 7.03e-4, 14.40e-4   # mm2 at 2 passes / 1 pass
FLOOR2 = 0.23e-4
C1_NS = 16 * 4 * 0.5 / 2.4       # PE ns/slot for one extra mm1 pass
C2_NS = 16 * 8 * 0.5 / 2.4

_kernel_cache: dict[tuple, object] = {}


def _t_tiles(C):
    tiles, t0 = [], 0
    while t0 < C:
        tt = TT if C - t0 >= TT else C - t0
        tiles.append((t0, tt))
        t0 += tt
    # Tail tile last: its cheap mm2 is the only un-overlapped one, and
    # full-size norm/GELU phases pair with full-size mm2 phases.
    return tiles


def _plan(u):
    """Per-slot pass levels (p, q) for mm1/mm2 given the normalized w^2 slot
    profile u, then snapped to per-tile compensation widths."""
    C = len(u)

    def plan_at(lam):
        p = np.full(C, 3, dtype=np.int64)
        q = np.full(C, 3, dtype=np.int64)
        p[u <= C1_NS / (lam * E1_2)] = 2
        p[(p == 2) & (u <= C1_NS / (lam * (E1_1 - E1_2)))] = 1
        q[u <= C2_NS / (lam * E2_2)] = 2
        q[(q == 2) & (u <= C2_NS / (lam * (E2_1 - E2_2)))] = 1
        return p, q

    def err2(p, q):
        t = np.zeros(C)
        t[p == 2] += E1_2
        t[p == 1] += E1_1
        t[q == 2] += E2_2
        t[q == 1] += E2_1
        return float((u * t).sum())

    B = ERR_TARGET * ERR_TARGET - FLOOR2
    lo, hi = 1e0, 1e14
    for _ in range(100):
        lam = (lo * hi) ** 0.5
        p, q = plan_at(lam)
        if err2(p, q) > B:
            lo = lam   # too much error -> fewer reductions needed
        else:
            hi = lam
    p, q = plan_at(hi)

    tiles = []
    for (t0, tt) in _t_tiles(C):
        def width(lev, need):
            n = int((lev[t0 : t0 + tt] >= need).sum())
            if n == 0:
                return 0
            n = min(tt, (n + 31) // 32 * 32)
            return max(n, min(tt, 128))
        g2, g3 = width(p, 2), width(p, 3)
        k2, k3 = width(q, 2), width(q, 3)
        tiles.append((tt, g2, g3, k2, k3))
    return tuple(tiles)


def _build(C: int, plan):
    f32, f32r, bf16, f8 = (
        mybir.dt.float32, mybir.dt.float32r, mybir.dt.bfloat16, mybir.dt.float8e4
    )
    DR = mybir.MatmulPerfMode.DoubleRow
    Mul, Add = mybir.AluOpType.mult, mybir.AluOpType.add
    nc = bacc.Bacc("TRN2", target_bir_lowering=False, debug=False, num_devices=8)
    XH = nc.dram_tensor("XH", [P, DP, 2, C], f8, kind="ExternalInput").ap()
    XL = nc.dram_tensor("XL", [P, DP, 2, C], f8, kind="ExternalInput").ap()
    W1H = nc.dram_tensor("W1H", [P, NJ, DP, 2, P], f8, kind="ExternalInput").ap()
    W1L = nc.dram_tensor("W1L", [P, NJ, DP, 2, P], f8, kind="ExternalInput").ap()
    W1SH = nc.dram_tensor("W1SH", [P, DP, 2, 32], f8, kind="ExternalInput").ap()
    SB1H = nc.dram_tensor("SB1H", [1, 1], f32, kind="ExternalInput").ap()
    W2H = nc.dram_tensor("W2H", [P, NK, KP, 2, P], f8, kind="ExternalInput").ap()
    W2L = nc.dram_tensor("W2L", [P, NK, KP, 2, P], f8, kind="ExternalInput").ap()
    YA = nc.dram_tensor("YA", [1, 1], f32, kind="ExternalInput").ap()
    YB = nc.dram_tensor("YB", [1, 1], f32, kind="ExternalInput").ap()
    b1 = nc.dram_tensor("b1", [P, NJ], f32, kind="ExternalInput").ap()
    b2 = nc.dram_tensor("b2", [P, NK], f32, kind="ExternalInput").ap()
    lg = nc.dram_tensor("lg", [P, NJ], f32, kind="ExternalInput").ap()
    lb = nc.dram_tensor("lb", [P, NJ], f32, kind="ExternalInput").ap()
    outT = nc.dram_tensor("outT", [P, NK, C], bf16, kind="ExternalOutput").ap()

    Gelu = mybir.ActivationFunctionType.Gelu
    Sqrt = mybir.ActivationFunctionType.Sqrt
    Ident = mybir.ActivationFunctionType.Identity

    with tile.TileContext(nc) as tc:
        with (
            tc.tile_pool(name="const", bufs=1) as constp,
            tc.tile_pool(name="wp", bufs=1) as wp,
            tc.tile_pool(name="xp", bufs=1) as xp,
            tc.tile_pool(name="hp", bufs=2) as hp,
            tc.tile_pool(name="hxp", bufs=2) as hxp,
            tc.tile_pool(name="sqp", bufs=2) as sqp,
            tc.tile_pool(name="op", bufs=2) as op,
            tc.tile_pool(name="statp", bufs=1) as statp,
            tc.tile_pool(name="ps_mm", bufs=6, space="PSUM") as ps_mm,
            tc.tile_pool(name="ps_acc", bufs=1, space="PSUM") as ps_acc,
        ):
            b1s = constp.tile([P, NJ], f32)
            b2s = constp.tile([P, NK], f32)
            lgs = constp.tile([P, NJ], f32)
            lbs = constp.tile([P, NJ], f32)
            w1sh = constp.tile([P, DP, 2, 32], f8)
            sb1h = constp.tile([1, 1], f32)
            ya = constp.tile([1, 1], f32)
            yb = constp.tile([1, 1], f32)

            def emit_const_dmas():
                nc.sync.dma_start(b1s[:], b1[:])
                nc.sync.dma_start(lgs[:], lg[:])
                nc.sync.dma_start(lbs[:], lb[:])
                nc.sync.dma_start(w1sh[:], W1SH[:])
                nc.sync.dma_start(sb1h[:], SB1H[:])
                nc.sync.dma_start(ya[:], YA[:])
                nc.sync.dma_start(yb[:], YB[:])

            ones_q = constp.tile([P, 2, 32], f8)   # lhsT for Q paired DR sums
            nc.any.memset(ones_q[:], 1.0)
            eps_t = constp.tile([1, 1], f32)
            nc.any.memset(eps_t[:], LN_EPS)

            # fp8 weights, SBUF-resident for the whole kernel, streamed in
            # contiguous per-chunk DMAs staged around the first two tiles.
            w1h = wp.tile([P, NJ, DP, 2, P], f8)
            w1l = wp.tile([P, NJ, DP, 2, P], f8)
            w2h = wp.tile([P, NK, KP, 2, P], f8)
            w2l = wp.tile([P, NK, KP, 2, P], f8)

            def emit_w1(a, b):
                nc.sync.dma_start(w1h[:, a:b], W1H[:, a:b])
                nc.sync.dma_start(w1l[:, a:b], W1L[:, a:b])

            def emit_w2(a, b):
                nc.sync.dma_start(w2h[:, a:b], W2H[:, a:b])
                nc.sync.dma_start(w2l[:, a:b], W2L[:, a:b])

            def emit_mm2(h_hi, h_lo, t0, tt, k2, k3, mid=None, post=None):
                # Graded fp8 mm2: full (w2h,h_hi) pass + (w2l,h_hi) over
                # [:k2] + (w2h,h_lo) over [:k3]; b2 added at the evict.
                # mid() runs after chain 5 (the next tile's stats-broadcast
                # matmuls); post(j) runs twice per chain from chain 6 (the
                # next tile's normalize/GELU work). With no post work (final
                # tile) the evict alternates DVE/ACT so neither throttles the
                # short chains.
                step = 0
                passes = [(w2h, h_hi, tt)]
                if k2:
                    passes.append((w2l, h_hi, k2))
                if k3:
                    passes.append((w2h, h_lo, k3))
                npass = len(passes)
                ot = None
                for k in range(NK):
                    pm = ps_mm.tile([P, TT], f32, tag="mm", name="mm2")[:, :tt]
                    for pi, (wt, ht, g) in enumerate(passes):
                        for kp in range(KP):
                            nc.tensor.matmul(
                                pm[:, :g],
                                wt[:, k, kp, :, :],
                                ht[:, kp, :, :g],
                                start=(pi == 0 and kp == 0),
                                stop=(pi == npass - 1 and kp == KP - 1),
                                perf_mode=DR,
                            )
                    if k % 2 == 0:
                        # batch 2 output chunks per DMA: halves the number of
                        # HWDGE acquisitions (the exclusive HWDGE device
                        # serializes the kernel drain otherwise)
                        ot = op.tile([P, 2, tt], bf16, tag=f"out{tt}",
                                     name="out", bufs=(2 if tt == TT else 5))
                    nc.vector.tensor_scalar(
                        ot[:, k % 2, :], pm[:], 1.0 / SW2, b2s[:, k : k + 1],
                        Mul, Add,
                    )
                    if k % 2 == 1:
                        nc.sync.dma_start(
                            outT[:, k - 1 : k + 1, t0 : t0 + tt], ot[:]
                        )
                    if k == 5 and mid is not None:
                        mid()
                    if k >= 6 and post is not None:
                        for _ in range(2):
                            if step < NJ:
                                post(step)
                                step += 1
                while post is not None and step < NJ:
                    post(step)
                    step += 1

            prev = None
            backlog = []
            tiles = _t_tiles(C)

            def emit_x(i):
                # Prefetch tile i's activations (one tile ahead of use) so
                # the DMA never queues behind an out-DMA whose SEQ wait only
                # clears at the end of an mm2 phase.
                t0, tt = tiles[i]
                g2 = plan[i][1]
                xh = xp.tile([P, DP, 2, TT], f8, tag="xh", name="xh", bufs=2)
                nc.sync.dma_start(xh[:, :, :, :tt], XH[:, :, :, t0 : t0 + tt])
                if g2:
                    xl = xp.tile([P, DP, 2, TT], f8, tag="xl", name="xl", bufs=2)
                    nc.sync.dma_start(xl[:, :, :, :g2], XL[:, :, :, t0 : t0 + g2])
                else:
                    xl = None
                return xh, xl

            x_pref = None
            for tile_i, (t0, tt) in enumerate(tiles):
                tt_, g2, g3, k2, k3 = plan[tile_i]
                assert tt_ == tt
                if tile_i == 0:
                    emit_w1(0, 1)  # W1 j=0 ahead of x so the first chain starts fast
                    x_pref = emit_x(0)
                xh, xl = x_pref
                if tile_i == 0:
                    emit_w1(1, 3)
                    emit_const_dmas()
                h = hp.tile(
                    [P, NJ, tt], bf16, tag=f"h{tt}", name="h",
                    bufs=(2 if tt == TT else 1),
                )
                h_hi = hxp.tile(
                    [P, KP, 2, tt], f8, tag=f"hh{tt}", name="h_hi",
                    bufs=(2 if tt == TT else 1),
                )
                h_lo = hxp.tile(
                    [P, KP, 2, tt], f8, tag=f"hl{tt}", name="h_lo",
                    bufs=(2 if tt == TT else 1),
                ) if k3 else None
                s_ps = ps_acc.tile([32, TT], f32, tag="sacc", name="sacc")[:, :tt]
                q_ps = ps_acc.tile([32, TT], f32, tag="qacc", name="qacc")[:, :tt]

                # ---- graded mm1; Q ones-matmuls deferred one pair so the PE
                # never waits on the ACT evict / DVE square chain; tile0's
                # norm/GELU backlog drip-fed through tile1's loop ----
                m1_passes = [(w1h, xh, tt)]
                if g2:
                    m1_passes.append((w1h, xl, g2))
                if g3:
                    m1_passes.append((w1l, xh, g3))
                np1 = len(m1_passes)
                pend_q = None
                sq = None
                for j in range(NJ):
                    if tile_i == 0:
                        if j == 0:
                            emit_w1(3, 8)
                        elif j == 4:
                            emit_w1(8, NJ)
                        elif j == 8:
                            emit_w2(0, 4)
                        elif j == 12:
                            emit_w2(4, 8)
                    elif tile_i == 1:
                        if j == 0:
                            emit_w2(8, 12)
                        elif j == 4:
                            emit_w2(12, NK)
                        elif j == 8:
                            nc.sync.dma_start(b2s[:], b2[:])
                    pm = ps_mm.tile([P, TT], f32, tag="mm", name="mm1")[:, :tt]
                    for pi, (wt, xt, g) in enumerate(m1_passes):
                        for dp in range(DP):
                            nc.tensor.matmul(
                                pm[:, :g],
                                wt[:, j, dp, :, :],
                                xt[:, dp, :, :g],
                                start=(pi == 0 and dp == 0),
                                stop=(pi == np1 - 1 and dp == DP - 1),
                                perf_mode=DR,
                            )
                    nc.scalar.activation(
                        h[:, j, :], pm[:], Ident,
                        bias=b1s[:, j : j + 1], scale=1.0 / (SX * SW1),
                    )
                    if j % 2 == 0:
                        sq = sqp.tile([P, 2, TT], f8, tag="sq", name="sq")
                    nc.vector.tensor_mul(sq[:, j % 2, :tt], h[:, j, :], h[:, j, :])
                    if j % 2 == 1:
                        if pend_q is not None:
                            jp, sqt = pend_q
                            nc.tensor.matmul(
                                q_ps[:], ones_q[:], sqt[:, :, :tt],
                                start=(jp == 0), stop=(jp == NJ // 2 - 1),
                                perf_mode=DR,
                            )
                        pend_q = (j // 2, sq)
                    if backlog:
                        backlog.pop(0)()
                while backlog:  # finish tile0's backlog before mm2(0) reads h
                    backlog.pop(0)()
                # S-fold: the W1-column-sum row, 1-pass DR into s_ps
                for dp in range(DP):
                    nc.tensor.matmul(
                        s_ps[:],
                        w1sh[:, dp, :, :],
                        xh[:, dp, :, :tt],
                        start=(dp == 0),
                        stop=(dp == DP - 1),
                        perf_mode=DR,
                    )
                jp, sqt = pend_q
                nc.tensor.matmul(
                    q_ps[:], ones_q[:], sqt[:, :, :tt],
                    start=(jp == 0), stop=(jp == NJ // 2 - 1),
                    perf_mode=DR,
                )

                # ---- LN stats (DVE/ACT only; broadcasts happen mid-mm2) ----
                mu = statp.tile([1, TT], f32, tag="mu", name="mu")[:, :tt]
                nc.vector.tensor_scalar(
                    mu[:], s_ps[0:1, :], 1.0 / (SX * SW1S * H), sb1h[:], Mul, Add
                )
                tmp = statp.tile([1, TT], f32, tag="tmp", name="tmp")[:, :tt]
                nc.vector.tensor_scalar_mul(tmp[:], q_ps[0:1, :], 1.0 / H)
                tmp2 = statp.tile([1, TT], f32, tag="tmp2", name="tmp2")[:, :tt]
                nc.vector.tensor_mul(tmp2[:], mu[:], mu[:])
                nc.vector.tensor_sub(tmp[:], tmp[:], tmp2[:])          # var
                nc.vector.tensor_scalar(tmp2[:], tmp[:], ya[:], yb[:], Mul, Add)
                nc.vector.tensor_mul(tmp[:], tmp[:], tmp2[:])
                nc.vector.tensor_mul(tmp[:], tmp[:], tmp2[:])
                nc.vector.tensor_scalar(tmp[:], tmp[:], -0.5, 1.5, Mul, Add)
                nc.vector.tensor_mul(tmp[:], tmp[:], tmp2[:])          # rstd
                a_row = statp.tile([1, TT], bf16, tag="a_row", name="a_row", bufs=2)
                nc.vector.tensor_copy(a_row[:, :tt], tmp[:])
                b_row = statp.tile([1, TT], bf16, tag="b_row", name="b_row", bufs=2)
                nc.vector.tensor_mul(b_row[:, :tt], mu[:], tmp[:])

                a_sb = statp.tile([P, TT], bf16, tag="a_sb", name="a_sb", bufs=2)
                b_sb = statp.tile([P, TT], bf16, tag="b_sb", name="b_sb", bufs=2)

                def emit_bc(a_row=a_row, b_row=b_row, a_sb=a_sb, b_sb=b_sb, tt=tt):
                    # per-token stat rows -> all partitions, on the idle
                    # GPSIMD engine (frees the PE matmuls, the DVE
                    # PSUM-copies, and two PSUM banks)
                    nc.gpsimd.partition_broadcast(a_sb[:, :tt], a_row[:, :tt])
                    nc.gpsimd.partition_broadcast(b_sb[:, :tt], b_row[:, :tt])

                def emit_norm_gelu(
                    j, h=h, h_hi=h_hi, h_lo=h_lo, a_sb=a_sb, b_sb=b_sb, tt=tt,
                    k3=k3, spread=False,
                ):
                    # normalize (DVE bf16) + GELU (ACT: fp8 h_hi full width;
                    # bf16 + h_lo residual only over [:k3]). In spread mode
                    # (tile0 backlog, no mm2 window to hide in) the h_lo path
                    # uses a Pool copy and alternating Pool/DVE subs so the
                    # work balances across all three engines.
                    jp, pl = j // 2, j % 2
                    hj = h[:, j, :tt]
                    nc.vector.tensor_mul(hj, hj, a_sb[:, :tt])
                    nc.vector.tensor_sub(hj, hj, b_sb[:, :tt])
                    if spread and (j % 8 == 7 or not k3):
                        # rebalance: these backlog items skip the Pool copy
                        # (Pool is the overloaded engine in the first tile's
                        # backlog window); gelu straight into h_hi on ACT
                        spread = False
                    if spread:
                        nc.scalar.activation(
                            hj, hj, Gelu,
                            bias=lbs[:, j : j + 1], scale=lgs[:, j : j + 1],
                        )
                        nc.gpsimd.tensor_copy(h_hi[:, jp, pl, :tt], hj)
                        if k3:
                            eng = nc.gpsimd if j % 2 else nc.vector
                            eng.tensor_sub(
                                h_lo[:, jp, pl, :k3], hj[:, :k3], h_hi[:, jp, pl, :k3]
                            )
                        return
                    nc.scalar.activation(
                        h_hi[:, jp, pl, :tt], hj, Gelu,
                        bias=lbs[:, j : j + 1], scale=lgs[:, j : j + 1],
                    )
                    if k3:
                        nc.scalar.activation(
                            hj[:, :k3], hj[:, :k3], Gelu,
                            bias=lbs[:, j : j + 1], scale=lgs[:, j : j + 1],
                        )
                        nc.vector.tensor_sub(
                            h_lo[:, jp, pl, :k3], hj[:, :k3], h_hi[:, jp, pl, :k3]
                        )

                if tile_i + 1 < len(tiles):
                    x_pref = emit_x(tile_i + 1)

                # ---- previous tile's mm2 on the PE, with this tile's
                # broadcasts at chain 5 and norm/GELU from chain 6 ----
                if prev is not None:
                    emit_mm2(*prev, mid=emit_bc, post=emit_norm_gelu)
                else:
                    backlog.append(emit_bc)
                    backlog.extend(
                        (lambda j=j, f=emit_norm_gelu: f(j, spread=True))
                        for j in range(NJ)
                    )
                prev = (h_hi, h_lo, t0, tt, k2, k3)

            if len(tiles) == 1:  # safety for tiny C: no tile-1 DMA slots
                emit_w2(8, NK)
                nc.sync.dma_start(b2s[:], b2[:])
            for fn in backlog:
                fn()
            emit_mm2(*prev)

    nc.compile()
    return nc


def _route(x64, Wg64, bg64):
    """Host gating: per-token top-2 expert ids and renormalized weights."""
    logits = x64 @ Wg64 + bg64                      # [N, E] fp64
    order = np.argsort(-logits, axis=1, kind="stable")[:, :TOPK]
    l0 = np.take_along_axis(logits, order, axis=1)  # [N, 2] descending
    w0 = 1.0 / (1.0 + np.exp(l0[:, 1] - l0[:, 0]))
    w = np.stack([w0, 1.0 - w0], axis=1)
    return order, w


def _split8(a):
    hi = a.astype(F8)
    lo = (a - hi.astype(np.float32)).astype(F8)
    return hi, lo


def kernel(x, W1, b1, ln_g, ln_b, W2, b2, Wg, bg):
    x = np.ascontiguousarray(np.asarray(x, dtype=np.float32))
    W1 = np.asarray(W1, dtype=np.float32)
    b1 = np.asarray(b1, dtype=np.float32)
    ln_g = np.asarray(ln_g, dtype=np.float32)
    ln_b = np.asarray(ln_b, dtype=np.float32)
    W2 = np.asarray(W2, dtype=np.float32)
    b2 = np.asarray(b2, dtype=np.float32)
    Wg = np.asarray(Wg, dtype=np.float32)
    bg = np.asarray(bg, dtype=np.float32)
    N = x.shape[0]

    order, w = _route(x.astype(np.float64), Wg.astype(np.float64), bg.astype(np.float64))

    tok_idx, tok_w = [], []
    for e in range(E):
        sel = np.nonzero((order[:, 0] == e) | (order[:, 1] == e))[0]
        we = np.where(order[sel, 0] == e, w[sel, 0], w[sel, 1]).astype(np.float32)
        o = np.argsort(-we, kind="stable")   # high-combine-weight slots first
        tok_idx.append(sel[o])
        tok_w.append(we[o])
    C = max(GRAN, int(-(-max(len(s) for s in tok_idx) // GRAN)) * GRAN)

    # normalized w^2 slot profile -> graded pass plan (shared by all cores)
    u = np.zeros(C)
    for e in range(E):
        u[: len(tok_w[e])] += tok_w[e].astype(np.float64) ** 2
    u /= u.sum()
    plan = _plan(u)

    key = (C, plan)
    if key not in _kernel_cache:
        _kernel_cache[key] = _build(C, plan)
    nc = _kernel_cache[key]

    in_maps = []
    for e in range(E):
        idx = np.zeros(C, dtype=np.int64)
        idx[: len(tok_idx[e])] = tok_idx[e]
        xg = x[idx] * SX                              # [C, D]
        xg[len(tok_idx[e]):] = 0.0
        xh, xl = _split8(xg)
        # [C, D] -> [P, DP, 2, C]
        xh_d = np.ascontiguousarray(xh.reshape(C, DP, 2, P).transpose(3, 1, 2, 0))
        xl_d = np.ascontiguousarray(xl.reshape(C, DP, 2, P).transpose(3, 1, 2, 0))
        w1h, w1l = _split8(W1[e] * SW1)               # [D, H]
        w1h_d = np.ascontiguousarray(
            w1h.reshape(DP, 2, P, NJ, P).transpose(2, 3, 0, 1, 4)
        )
        w1l_d = np.ascontiguousarray(
            w1l.reshape(DP, 2, P, NJ, P).transpose(2, 3, 0, 1, 4)
        )
        # S-fold: column-sum of W1 (scaled), replicated over 32 lhsT columns
        w1s = W1[e].sum(axis=1) * SW1S                # [D]
        w1sh, _ = _split8(w1s)
        w1sh_d = np.ascontiguousarray(np.broadcast_to(
            w1sh.reshape(DP, 2, P).transpose(2, 0, 1)[:, :, :, None], (P, DP, 2, 32)
        ).astype(F8))
        sb1h_d = np.full((1, 1), b1[e].sum() / H, dtype=np.float32)
        w1c = W1[e] - W1[e].mean(axis=1, keepdims=True)
        vbar = float((w1c * w1c).sum() / H + np.var(b1[e]))
        y0 = 1.0 / np.sqrt(vbar + LN_EPS)
        ya_d = np.full((1, 1), -0.5 * y0 ** 3, dtype=np.float32)
        yb_d = np.full((1, 1), 1.5 * y0, dtype=np.float32)
        w2h, w2l = _split8(W2[e] * SW2)               # [H, H]
        w2h_d = np.ascontiguousarray(
            w2h.reshape(KP, 2, P, NK, P).transpose(2, 3, 0, 1, 4)
        )
        w2l_d = np.ascontiguousarray(
            w2l.reshape(KP, 2, P, NK, P).transpose(2, 3, 0, 1, 4)
        )
        in_maps.append(
            {
                "XH": xh_d,
                "XL": xl_d,
                "W1H": w1h_d,
                "W1L": w1l_d,
                "W1SH": w1sh_d,
                "SB1H": sb1h_d,
                "YA": ya_d,
                "YB": yb_d,
                "W2H": w2h_d,
                "W2L": w2l_d,
                "b1": np.ascontiguousarray(b1[e].reshape(NJ, P).T),
                "b2": np.ascontiguousarray(b2[e].reshape(NK, P).T),
                "lg": np.ascontiguousarray(ln_g[e].reshape(NJ, P).T),
                "lb": np.ascontiguousarray(ln_b[e].reshape(NJ, P).T),
            }
        )

    results = _run(key, nc, in_maps)

    y = np.zeros((N, H), dtype=np.float32)
    for e in range(E):
        cnt = len(tok_idx[e])
        eoT = (
            results[e]["outT"].transpose(1, 0, 2).reshape(H, C).astype(np.float32)
        )
        y[tok_idx[e]] += tok_w[e][:, None] * eoT[:, :cnt].T
    return y


_neff_cache: dict[tuple, str] = {}


def _run(key, nc, in_maps):
    C = key[0]
    if axon_active():
        # PJRT path; NEFF compile is cached by libneuronxla.
        return run_bass_kernel_spmd(nc, in_maps, core_ids=list(range(E))).results
    # Native path: compile once per capacity, then execute the cached NEFF.
    from concourse.bass_utils import compile_bass_kernel, run_neff

    if key not in _neff_cache:
        _neff_cache[key] = compile_bass_kernel(nc, tempfile.mkdtemp())
    out_maps = [{"outT": np.zeros((P, NK, C), dtype=BF)} for _ in range(E)]
    in_maps = [m.copy() for m in in_maps]
    if nc.partition_id_tensor:
        for core_id, m in enumerate(in_maps):
            m[nc.partition_id_tensor.name] = np.array([[core_id]], dtype=np.uint32)
    return run_neff(
        _neff_cache[key],
        in_maps,
        out_maps,
        core_ids=list(range(E)),
        has_collectives=False,
    )


# revision 7
# speedup vs baseline: 1.1959x; 1.0106x over previous
# MoE (top-2 of 8 experts) kernel for 8 Trainium2 NeuronCores.
#
# Strategy: expert-parallel sparse routing with fp8 DoubleRow matmuls and
# GRADED PRECISION. Host computes the gating network and per-expert token
# lists; core e runs expert e's FFN (x@W1+b1 -> LayerNorm -> erf-GELU ->
# @W2+b2) on its routed tokens, sorted by combine weight (descending).
# Matmuls run as fp8(e4m3) DoubleRow with hi/lo error compensation, but the
# compensation passes cover only a column prefix of each (w-sorted) token
# tile: tokens with large combine weights get full 3-pass accuracy, tokens
# with small weights (and padding) get 1-2 passes. The per-slot pass plan is
# chosen by a Lagrangian knapsack on the w^2 profile so the end-to-end
# rel err stays under a fixed budget while minimizing PE cycles.
# All weights live in SBUF (10MB fp8), loaded once in contiguous per-chunk
# DMAs. LayerNorm S-sums come from a 1-pass W1-column-sum lhsT row; Q-sums
# use a paired-fp8 DoubleRow ones-matmul on h^2; per-token stats broadcast
# across partitions with K=1 f32r matmuls, emitted mid-mm2 so the PE never
# waits on the stats chain. The first tile's normalize/GELU backlog is
# drip-fed through the second tile's mm1 loop.

import tempfile

import ml_dtypes
import numpy as np

import concourse.bacc as bacc
import concourse.mybir as mybir
import concourse.tile as tile
from concourse._compat import axon_active
from concourse.bass_utils import run_bass_kernel_spmd

P = 128
D, H, E, TOPK = 1024, 2048, 8, 2
DP, KP, NJ, NK = D // 256, H // 256, H // P, H // P  # 4, 8, 16, 16
LN_EPS = 1e-5
TT = 512           # main token tile
GRAN = 16          # capacity granularity
SX, SW1, SW2 = 16.0, 256.0, 256.0   # fp8 pre-quantization scales
SW1S = 32.0        # scale for the W1 column-sum row (S-fold)
F8 = ml_dtypes.float8_e4m3
BF = ml_dtypes.bfloat16

# Error model for the pass planner (err^2 contributions, measured on the
# reference input distribution; used as a heuristic for any input).
ERR_TARGET = 1.78e-2
E1_2, E1_1 = 5.98e-4, 13.49e-4   # mm1 at 2 passes / 1 pass
E2_2, E2_1 = 7.03e-4, 14.40e-4   # mm2 at 2 passes / 1 pass
FLOOR2 = 0.23e-4
C1_NS = 16 * 4 * 0.5 / 2.4       # PE ns/slot for one extra mm1 pass
C2_NS = 16 * 8 * 0.5 / 2.4

_kernel_cache: dict[tuple, object] = {}


def _t_tiles(C):
    tiles, t0 = [], 0
    while t0 < C:
        tt = TT if C - t0 >= TT else C - t0
        tiles.append((t0, tt))
        t0 += tt
    # Tail tile last: its cheap mm2 is the only un-overlapped one, and
    # full-size norm/GELU phases pair with full-size mm2 phases.
    return tiles


def _plan(u):
    """Per-slot pass levels (p, q) for mm1/mm2 given the normalized w^2 slot
    profile u, then snapped to per-tile compensation widths."""
    C = len(u)

    def plan_at(lam):
        p = np.full(C, 3, dtype=np.int64)
        q = np.full(C, 3, dtype=np.int64)
        p[u <= C1_NS / (lam * E1_2)] = 2
        p[(p == 2) & (u <= C1_NS / (lam * (E1_1 - E1_2)))] = 1
        q[u <= C2_NS / (lam * E2_2)] = 2
        q[(q == 2) & (u <= C2_NS / (lam * (E2_1 - E2_2)))] = 1
        return p, q

    def err2(p, q):
        t = np.zeros(C)
        t[p == 2] += E1_2
        t[p == 1] += E1_1
        t[q == 2] += E2_2
        t[q == 1] += E2_1
        return float((u * t).sum())

    B = ERR_TARGET * ERR_TARGET - FLOOR2
    lo, hi = 1e0, 1e14
    for _ in range(100):
        lam = (lo * hi) ** 0.5
        p, q = plan_at(lam)
        if err2(p, q) > B:
            lo = lam   # too much error -> fewer reductions needed
        else:
            hi = lam
    p, q = plan_at(hi)

    tiles = []
    for (t0, tt) in _t_tiles(C):
        def width(lev, need):
            n = int((lev[t0 : t0 + tt] >= need).sum())
            if n == 0:
                return 0
            n = min(tt, (n + 31) // 32 * 32)
            return max(n, min(tt, 128))
        g2, g3 = width(p, 2), width(p, 3)
        k2, k3 = width(q, 2), width(q, 3)
        tiles.append((tt, g2, g3, k2, k3))
    return tuple(tiles)


def _build(C: int, plan):
    f32, f32r, bf16, f8 = (
        mybir.dt.float32, mybir.dt.float32r, mybir.dt.bfloat16, mybir.dt.float8e4
    )
    DR = mybir.MatmulPerfMode.DoubleRow
    Mul, Add = mybir.AluOpType.mult, mybir.AluOpType.add
    nc = bacc.Bacc("TRN2", target_bir_lowering=False, debug=False, num_devices=8)
    XH = nc.dram_tensor("XH", [P, DP, 2, C], f8, kind="ExternalInput").ap()
    XL = nc.dram_tensor("XL", [P, DP, 2, C], f8, kind="ExternalInput").ap()
    W1H = nc.dram_tensor("W1H", [P, NJ, DP, 2, P], f8, kind="ExternalInput").ap()
    W1L = nc.dram_tensor("W1L", [P, NJ, DP, 2, P], f8, kind="ExternalInput").ap()
    W1SH = nc.dram_tensor("W1SH", [P, DP, 2, 32], f8, kind="ExternalInput").ap()
    SB1H = nc.dram_tensor("SB1H", [1, 1], f32, kind="ExternalInput").ap()
    W2H = nc.dram_tensor("W2H", [P, NK, KP, 2, P], f8, kind="ExternalInput").ap()
    W2L = nc.dram_tensor("W2L", [P, NK, KP, 2, P], f8, kind="ExternalInput").ap()
    YA = nc.dram_tensor("YA", [1, 1], f32, kind="ExternalInput").ap()
    YB = nc.dram_tensor("YB", [1, 1], f32, kind="ExternalInput").ap()
    b1 = nc.dram_tensor("b1", [P, NJ], f32, kind="ExternalInput").ap()
    b2 = nc.dram_tensor("b2", [P, NK], f32, kind="ExternalInput").ap()
    lg = nc.dram_tensor("lg", [P, NJ], f32, kind="ExternalInput").ap()
    lb = nc.dram_tensor("lb", [P, NJ], f32, kind="ExternalInput").ap()
    outT = nc.dram_tensor("outT", [P, NK, C], bf16, kind="ExternalOutput").ap()

    Gelu = mybir.ActivationFunctionType.Gelu
    Sqrt = mybir.ActivationFunctionType.Sqrt
    Ident = mybir.ActivationFunctionType.Identity

    with tile.TileContext(nc) as tc:
        with (
            tc.tile_pool(name="const", bufs=1) as constp,
            tc.tile_pool(name="wp", bufs=1) as wp,
            tc.tile_pool(name="xp", bufs=1) as xp,
            tc.tile_pool(name="hp", bufs=2) as hp,
            tc.tile_pool(name="hxp", bufs=2) as hxp,
            tc.tile_pool(name="sqp", bufs=2) as sqp,
            tc.tile_pool(name="op", bufs=2) as op,
            tc.tile_pool(name="statp", bufs=1) as statp,
            tc.tile_pool(name="ps_mm", bufs=6, space="PSUM") as ps_mm,
            tc.tile_pool(name="ps_acc", bufs=1, space="PSUM") as ps_acc,
        ):
            b1s = constp.tile([P, NJ], f32)
            b2s = constp.tile([P, NK], f32)
            lgs = constp.tile([P, NJ], f32)
            lbs = constp.tile([P, NJ], f32)
            w1sh = constp.tile([P, DP, 2, 32], f8)
            sb1h = constp.tile([1, 1], f32)
            ya = constp.tile([1, 1], f32)
            yb = constp.tile([1, 1], f32)

            def emit_const_dmas():
                nc.sync.dma_start(b1s[:], b1[:])
                nc.sync.dma_start(lgs[:], lg[:])
                nc.sync.dma_start(lbs[:], lb[:])
                nc.sync.dma_start(w1sh[:], W1SH[:])
                nc.sync.dma_start(sb1h[:], SB1H[:])
                nc.sync.dma_start(ya[:], YA[:])
                nc.sync.dma_start(yb[:], YB[:])

            ones_q = constp.tile([P, 2, 32], f8)   # lhsT for Q paired DR sums
            nc.any.memset(ones_q[:], 1.0)
            eps_t = constp.tile([1, 1], f32)
            nc.any.memset(eps_t[:], LN_EPS)

            # fp8 weights, SBUF-resident for the whole kernel, streamed in
            # contiguous per-chunk DMAs staged around the first two tiles.
            w1h = wp.tile([P, NJ, DP, 2, P], f8)
            w1l = wp.tile([P, NJ, DP, 2, P], f8)
            w2h = wp.tile([P, NK, KP, 2, P], f8)
            w2l = wp.tile([P, NK, KP, 2, P], f8)

            def emit_w1(a, b):
                nc.sync.dma_start(w1h[:, a:b], W1H[:, a:b])
                nc.sync.dma_start(w1l[:, a:b], W1L[:, a:b])

            def emit_w2(a, b):
                nc.sync.dma_start(w2h[:, a:b], W2H[:, a:b])
                nc.sync.dma_start(w2l[:, a:b], W2L[:, a:b])

            def emit_mm2(h_hi, h_lo, t0, tt, k2, k3, mid=None, post=None):
                # Graded fp8 mm2: full (w2h,h_hi) pass + (w2l,h_hi) over
                # [:k2] + (w2h,h_lo) over [:k3]; b2 added at the evict.
                # mid() runs after chain 5 (the next tile's stats-broadcast
                # matmuls); post(j) runs twice per chain from chain 6 (the
                # next tile's normalize/GELU work). With no post work (final
                # tile) the evict alternates DVE/ACT so neither throttles the
                # short chains.
                step = 0
                passes = [(w2h, h_hi, tt)]
                if k2:
                    passes.append((w2l, h_hi, k2))
                if k3:
                    passes.append((w2h, h_lo, k3))
                npass = len(passes)
                ot = None
                for k in range(NK):
                    pm = ps_mm.tile([P, TT], f32, tag="mm", name="mm2")[:, :tt]
                    for pi, (wt, ht, g) in enumerate(passes):
                        for kp in range(KP):
                            nc.tensor.matmul(
                                pm[:, :g],
                                wt[:, k, kp, :, :],
                                ht[:, kp, :, :g],
                                start=(pi == 0 and kp == 0),
                                stop=(pi == npass - 1 and kp == KP - 1),
                                perf_mode=DR,
                            )
                    if k % 2 == 0:
                        # batch 2 output chunks per DMA: halves the number of
                        # HWDGE acquisitions (the exclusive HWDGE device
                        # serializes the kernel drain otherwise)
                        ot = op.tile([P, 2, tt], bf16, tag=f"out{tt}",
                                     name="out", bufs=(2 if tt == TT else 5))
                    nc.vector.tensor_scalar(
                        ot[:, k % 2, :], pm[:], 1.0 / SW2, b2s[:, k : k + 1],
                        Mul, Add,
                    )
                    if k % 2 == 1:
                        nc.sync.dma_start(
                            outT[:, k - 1 : k + 1, t0 : t0 + tt], ot[:]
                        )
                    if k == 5 and mid is not None:
                        mid()
                    if k >= 6 and post is not None:
                        for _ in range(2):
                            if step < NJ:
                                post(step)
                                step += 1
                while post is not None and step < NJ:
                    post(step)
                    step += 1

            prev = None
            backlog = []
            tiles = _t_tiles(C)

            def emit_x(i):
                # Prefetch tile i's activations (one tile ahead of use) so
                # the DMA never queues behind an out-DMA whose SEQ wait only
                # clears at the end of an mm2 phase.
                t0, tt = tiles[i]
                g2 = plan[i][1]
                xh = xp.tile([P, DP, 2, TT], f8, tag="xh", name="xh", bufs=2)
                nc.sync.dma_start(xh[:, :, :, :tt], XH[:, :, :, t0 : t0 + tt])
                if g2:
                    xl = xp.tile([P, DP, 2, TT], f8, tag="xl", name="xl", bufs=2)
                    nc.sync.dma_start(xl[:, :, :, :g2], XL[:, :, :, t0 : t0 + g2])
                else:
                    xl = None
                return xh, xl

            x_pref = None
            for tile_i, (t0, tt) in enumerate(tiles):
                tt_, g2, g3, k2, k3 = plan[tile_i]
                assert tt_ == tt
                if tile_i == 0:
                    emit_w1(0, 1)  # W1 j=0 ahead of x so the first chain starts fast
                    x_pref = emit_x(0)
                xh, xl = x_pref
                if tile_i == 0:
                    emit_w1(1, 3)
                    emit_const_dmas()
                h = hp.tile(
                    [P, NJ, tt], bf16, tag=f"h{tt}", name="h",
                    bufs=(2 if tt == TT else 1),
                )
                h_hi = hxp.tile(
                    [P, KP, 2, tt], f8, tag=f"hh{tt}", name="h_hi",
                    bufs=(2 if tt == TT else 1),
                )
                h_lo = hxp.tile(
                    [P, KP, 2, tt], f8, tag=f"hl{tt}", name="h_lo",
                    bufs=(2 if tt == TT else 1),
                ) if k3 else None
                s_ps = ps_acc.tile([32, TT], f32, tag="sacc", name="sacc")[:, :tt]
                q_ps = ps_acc.tile([32, TT], f32, tag="qacc", name="qacc")[:, :tt]

                # ---- graded mm1; Q ones-matmuls deferred one pair so the PE
                # never waits on the ACT evict / DVE square chain; tile0's
                # norm/GELU backlog drip-fed through tile1's loop ----
                m1_passes = [(w1h, xh, tt)]
                if g2:
                    m1_passes.append((w1h, xl, g2))
                if g3:
                    m1_passes.append((w1l, xh, g3))
                np1 = len(m1_passes)
                pend_q = None
                sq = None
                for j in range(NJ):
                    if tile_i == 0:
                        if j == 0:
                            emit_w1(3, 8)
                        elif j == 4:
                            emit_w1(8, NJ)
                        elif j == 8:
                            emit_w2(0, 4)
                        elif j == 12:
                            emit_w2(4, 8)
                    elif tile_i == 1:
                        if j == 0:
                            emit_w2(8, 12)
                        elif j == 4:
                            emit_w2(12, NK)
                        elif j == 8:
                            nc.sync.dma_start(b2s[:], b2[:])
                    pm = ps_mm.tile([P, TT], f32, tag="mm", name="mm1")[:, :tt]
                    for pi, (wt, xt, g) in enumerate(m1_passes):
                        for dp in range(DP):
                            nc.tensor.matmul(
                                pm[:, :g],
                                wt[:, j, dp, :, :],
                                xt[:, dp, :, :g],
                                start=(pi == 0 and dp == 0),
                                stop=(pi == np1 - 1 and dp == DP - 1),
                                perf_mode=DR,
                            )
                    nc.scalar.activation(
                        h[:, j, :], pm[:], Ident,
                        bias=b1s[:, j : j + 1], scale=1.0 / (SX * SW1),
                    )
                    if j % 2 == 0:
                        sq = sqp.tile([P, 2, TT], f8, tag="sq", name="sq")
                    nc.vector.tensor_mul(sq[:, j % 2, :tt], h[:, j, :], h[:, j, :])
                    if j % 2 == 1:
                        if pend_q is not None:
                            jp, sqt = pend_q
                            nc.tensor.matmul(
                                q_ps[:], ones_q[:], sqt[:, :, :tt],
                                start=(jp == 0), stop=(jp == NJ // 2 - 1),
                                perf_mode=DR,
                            )
                        pend_q = (j // 2, sq)
                    if backlog:
                        backlog.pop(0)()
                while backlog:  # finish tile0's backlog before mm2(0) reads h
                    backlog.pop(0)()
                # S-fold: the W1-column-sum row, 1-pass DR into s_ps
                for dp in range(DP):
                    nc.tensor.matmul(
                        s_ps[:],
                        w1sh[:, dp, :, :],
                        xh[:, dp, :, :tt],
                        start=(dp == 0),
                        stop=(dp == DP - 1),
                        perf_mode=DR,
                    )
                jp, sqt = pend_q
                nc.tensor.matmul(
                    q_ps[:], ones_q[:], sqt[:, :, :tt],
                    start=(jp == 0), stop=(jp == NJ // 2 - 1),
                    perf_mode=DR,
                )

                # ---- LN stats (DVE/ACT only; broadcasts happen mid-mm2) ----
                mu = statp.tile([1, TT], f32, tag="mu", name="mu")[:, :tt]
                nc.vector.tensor_scalar(
                    mu[:], s_ps[0:1, :], 1.0 / (SX * SW1S * H), sb1h[:], Mul, Add
                )
                tmp = statp.tile([1, TT], f32, tag="tmp", name="tmp")[:, :tt]
                nc.vector.tensor_scalar_mul(tmp[:], q_ps[0:1, :], 1.0 / H)
                tmp2 = statp.tile([1, TT], f32, tag="tmp2", name="tmp2")[:, :tt]
                nc.vector.tensor_mul(tmp2[:], mu[:], mu[:])
                nc.vector.tensor_sub(tmp[:], tmp[:], tmp2[:])          # var
                nc.vector.tensor_scalar(tmp2[:], tmp[:], ya[:], yb[:], Mul, Add)
                nc.vector.tensor_mul(tmp[:], tmp[:], tmp2[:])
                nc.vector.tensor_mul(tmp[:], tmp[:], tmp2[:])
                nc.vector.tensor_scalar(tmp[:], tmp[:], -0.5, 1.5, Mul, Add)
                nc.vector.tensor_mul(tmp[:], tmp[:], tmp2[:])          # rstd
                a_row = statp.tile([1, TT], bf16, tag="a_row", name="a_row", bufs=2)
                nc.vector.tensor_copy(a_row[:, :tt], tmp[:])
                b_row = statp.tile([1, TT], bf16, tag="b_row", name="b_row", bufs=2)
                nc.vector.tensor_mul(b_row[:, :tt], mu[:], tmp[:])

                a_sb = statp.tile([P, TT], bf16, tag="a_sb", name="a_sb", bufs=2)
                b_sb = statp.tile([P, TT], bf16, tag="b_sb", name="b_sb", bufs=2)

                def emit_bc(a_row=a_row, b_row=b_row, a_sb=a_sb, b_sb=b_sb, tt=tt):
                    # per-token stat rows -> all partitions, on the idle
                    # GPSIMD engine (frees the PE matmuls, the DVE
                    # PSUM-copies, and two PSUM banks)
                    nc.gpsimd.partition_broadcast(a_sb[:, :tt], a_row[:, :tt])
                    nc.gpsimd.partition_broadcast(b_sb[:, :tt], b_row[:, :tt])

                def emit_norm_gelu(
                    j, h=h, h_hi=h_hi, h_lo=h_lo, a_sb=a_sb, b_sb=b_sb, tt=tt,
                    k3=k3, spread=False,
                ):
                    # normalize (DVE bf16) + GELU (ACT: fp8 h_hi full width;
                    # bf16 + h_lo residual only over [:k3]). In spread mode
                    # (tile0 backlog, no mm2 window to hide in) the h_lo path
                    # uses a Pool copy and alternating Pool/DVE subs so the
                    # work balances across all three engines.
                    jp, pl = j // 2, j % 2
                    hj = h[:, j, :tt]
                    nc.vector.tensor_mul(hj, hj, a_sb[:, :tt])
                    nc.vector.tensor_sub(hj, hj, b_sb[:, :tt])
                    if spread and (j % 8 == 7 or not k3):
                        # rebalance: these backlog items skip the Pool copy
                        # (Pool is the overloaded engine in the first tile's
                        # backlog window); gelu straight into h_hi on ACT
                        spread = False
                    if spread:
                        nc.scalar.activation(
                            hj, hj, Gelu,
                            bias=lbs[:, j : j + 1], scale=lgs[:, j : j + 1],
                        )
                        nc.gpsimd.tensor_copy(h_hi[:, jp, pl, :tt], hj)
                        if k3:
                            eng = nc.gpsimd if j % 2 else nc.vector
                            eng.tensor_sub(
                                h_lo[:, jp, pl, :k3], hj[:, :k3], h_hi[:, jp, pl, :k3]
                            )
                        return
                    nc.scalar.activation(
                        h_hi[:, jp, pl, :tt], hj, Gelu,
                        bias=lbs[:, j : j + 1], scale=lgs[:, j : j + 1],
                    )
                    if k3:
                        nc.scalar.activation(
                            hj[:, :k3], hj[:, :k3], Gelu,
                            bias=lbs[:, j : j + 1], scale=lgs[:, j : j + 1],
                        )
                        nc.vector.tensor_sub(
                            h_lo[:, jp, pl, :k3], hj[:, :k3], h_hi[:, jp, pl, :k3]
                        )

                if tile_i + 1 < len(tiles):
                    x_pref = emit_x(tile_i + 1)

                # ---- previous tile's mm2 on the PE, with this tile's
                # broadcasts at chain 5 and norm/GELU from chain 6 ----
                if prev is not None:
                    emit_mm2(*prev, mid=emit_bc, post=emit_norm_gelu)
                else:
                    backlog.append(emit_bc)
                    backlog.extend(
                        (lambda j=j, f=emit_norm_gelu: f(j, spread=True))
                        for j in range(NJ)
                    )
                prev = (h_hi, h_lo, t0, tt, k2, k3)

            if len(tiles) == 1:  # safety for tiny C: no tile-1 DMA slots
                emit_w2(8, NK)
                nc.sync.dma_start(b2s[:], b2[:])
            for fn in backlog:
                fn()
            emit_mm2(*prev)

    nc.compile()
    return nc


def _route(x64, Wg64, bg64):
    """Host gating: per-token top-2 expert ids and renormalized weights."""
    logits = x64 @ Wg64 + bg64                      # [N, E] fp64
    order = np.argsort(-logits, axis=1, kind="stable")[:, :TOPK]
    l0 = np.take_along_axis(logits, order, axis=1)  # [N, 2] descending
    w0 = 1.0 / (1.0 + np.exp(l0[:, 1] - l0[:, 0]))
    w = np.stack([w0, 1.0 - w0], axis=1)
    return order, w


def _split8(a):
    hi = a.astype(F8)
    lo = (a - hi.astype(np.float32)).astype(F8)
    return hi, lo


def kernel(x, W1, b1, ln_g, ln_b, W2, b2, Wg, bg):
    x = np.ascontiguousarray(np.asarray(x, dtype=np.float32))
    W1 = np.asarray(W1, dtype=np.float32)
    b1 = np.asarray(b1, dtype=np.float32)
    ln_g = np.asarray(ln_g, dtype=np.float32)
    ln_b = np.asarray(ln_b, dtype=np.float32)
    W2 = np.asarray(W2, dtype=np.float32)
    b2 = np.asarray(b2, dtype=np.float32)
    Wg = np.asarray(Wg, dtype=np.float32)
    bg = np.asarray(bg, dtype=np.float32)
    N = x.shape[0]

    order, w = _route(x.astype(np.float64), Wg.astype(np.float64), bg.astype(np.float64))

    tok_idx, tok_w = [], []
    for e in range(E):
        sel = np.nonzero((order[:, 0] == e) | (order[:, 1] == e))[0]
        we = np.where(order[sel, 0] == e, w[sel, 0], w[sel, 1]).astype(np.float32)
        o = np.argsort(-we, kind="stable")   # high-combine-weight slots first
        tok_idx.append(sel[o])
        tok_w.append(we[o])
    C = max(GRAN, int(-(-max(len(s) for s in tok_idx) // GRAN)) * GRAN)

    # normalized w^2 slot profile -> graded pass plan (shared by all cores)
    u = np.zeros(C)
    for e in range(E):
        u[: len(tok_w[e])] += tok_w[e].astype(np.float64) ** 2
    u /= u.sum()
    plan = _plan(u)

    key = (C, plan)
    if key not in _kernel_cache:
        _kernel_cache[key] = _build(C, plan)
    nc = _kernel_cache[key]

    in_maps = []
    for e in range(E):
        idx = np.zeros(C, dtype=np.int64)
        idx[: len(tok_idx[e])] = tok_idx[e]
        xg = x[idx] * SX                              # [C, D]
        xg[len(tok_idx[e]):] = 0.0
        xh, xl = _split8(xg)
        # [C, D] -> [P, DP, 2, C]
        xh_d = np.ascontiguousarray(xh.reshape(C, DP, 2, P).transpose(3, 1, 2, 0))
        xl_d = np.ascontiguousarray(xl.reshape(C, DP, 2, P).transpose(3, 1, 2, 0))
        w1h, w1l = _split8(W1[e] * SW1)               # [D, H]
        w1h_d = np.ascontiguousarray(
            w1h.reshape(DP, 2, P, NJ, P).transpose(2, 3, 0, 1, 4)
        )
        w1l_d = np.ascontiguousarray(
            w1l.reshape(DP, 2, P, NJ, P).transpose(2, 3, 0, 1, 4)
        )
        # S-fold: column-sum of W1 (scaled), replicated over 32 lhsT columns
        w1s = W1[e].sum(axis=1) * SW1S                # [D]
        w1sh, _ = _split8(w1s)
        w1sh_d = np.ascontiguousarray(np.broadcast_to(
            w1sh.reshape(DP, 2, P).transpose(2, 0, 1)[:, :, :, None], (P, DP, 2, 32)
        ).astype(F8))
        sb1h_d = np.full((1, 1), b1[e].sum() / H, dtype=np.float32)
        w1c = W1[e] - W1[e].mean(axis=1, keepdims=True)
        vbar = float((w1c * w1c).sum() / H + np.var(b1[e]))
        y0 = 1.0 / np.sqrt(vbar + LN_EPS)
        ya_d = np.full((1, 1), -0.5 * y0 ** 3, dtype=np.float32)
        yb_d = np.full((1, 1), 1.5 * y0, dtype=np.float32)
        w2h, w2l = _split8(W2[e] * SW2)               # [H, H]
        w2h_d = np.ascontiguousarray(
            w2h.reshape(KP, 2, P, NK, P).transpose(2, 3, 0, 1, 4)
        )
        w2l_d = np.ascontiguousarray(
            w2l.reshape(KP, 2, P, NK, P).transpose(2, 3, 0, 1, 4)
        )
        in_maps.append(
            {
                "XH": xh_d,
                "XL": xl_d,
                "W1H": w1h_d,
                "W1L": w1l_d,
                "W1SH": w1sh_d,
                "SB1H": sb1h_d,
                "YA": ya_d,
                "YB": yb_d,
                "W2H": w2h_d,
                "W2L": w2l_d,
                "b1": np.ascontiguousarray(b1[e].reshape(NJ, P).T),
                "b2": np.ascontiguousarray(b2[e].reshape(NK, P).T),
                "lg": np.ascontiguousarray(ln_g[e].reshape(NJ, P).T),
                "lb": np.ascontiguousarray(ln_b[e].reshape(NJ, P).T),
            }
        )

    results = _run(key, nc, in_maps)

    y = np.zeros((N, H), dtype=np.float32)
    for e in range(E):
        cnt = len(tok_idx[e])
        eoT = (
            results[e]["outT"].transpose(1, 0, 2).reshape(H, C).astype(np.float32)
        )
        y[tok_idx[e]] += tok_w[e][:, None] * eoT[:, :cnt].T
    return y


_neff_cache: dict[tuple, str] = {}


def _run(key, nc, in_maps):
    C = key[0]
    if axon_active():
        # PJRT path; NEFF compile is cached by libneuronxla.
        return run_bass_kernel_spmd(nc, in_maps, core_ids=list(range(E))).results
    # Native path: compile once per capacity, then execute the cached NEFF.
    from concourse.bass_utils import compile_bass_kernel, run_neff

    if key not in _neff_cache:
        _neff_cache[key] = compile_bass_kernel(nc, tempfile.mkdtemp())
    out_maps = [{"outT": np.zeros((P, NK, C), dtype=BF)} for _ in range(E)]
    in_maps = [m.copy() for m in in_maps]
    if nc.partition_id_tensor:
        for core_id, m in enumerate(in_maps):
            m[nc.partition_id_tensor.name] = np.array([[core_id]], dtype=np.uint32)
    return run_neff(
        _neff_cache[key],
        in_maps,
        out_maps,
        core_ids=list(range(E)),
        has_collectives=False,
    )
